# revision 1
# baseline (speedup 1.0000x reference)
"""Trainium2 Bass kernel for stacked-LSTM + attention + dense head (v2).

Model (per reference):
  3x LSTM(H=512, return_sequences) with inference BatchNorm between layers,
  attention pooling over time, then Dense(128)+BN+Dense(64)+Dense(5).
  B=128, T=512, D=128, H=512, fp32.

Strategy: data-parallel over batch (16 rows/core on 8 cores). Per core a
3-layer wavefront with lags 16/32 (ring-aligned). Key structure vs v1:
  - xz (input projection) is INJECTED into PSUM by K=16 identity matmuls
    (plus the recurrent matmuls accumulate on top), removing the wide DVE
    adds from the critical chain.
  - Biases enter via K=1 ones-row matmuls (no DVE bias adds anywhere).
  - Gates use an interleaved column permutation [i_n|f_n|o_n|g_n] per
    512-chunk so sigmoid/tanh run per-chunk, hidden under the MM stream.
  - Projections for layers 1/2 run on PE column strip 3 (2 steps/tile),
    evicted by a single tensor_copy and DMA'd into a 16-slot SBUF ring.
  - The c/h elementwise chain is split across DVE and GpSimd.
  - h transpose via DVE StreamTranspose + 12 small scatter DMAs spread
    over 4 issuing engines.

Self-contained: hardcodes shapes; no reads of reference.py/spec.json.
"""

import functools

import numpy as np

B, T, D, H = 128, 512, 128, 512
NC = 8
BL = B // NC          # batch rows per core
G4 = 4 * H            # gate width 2048
EPS = 1e-3
P = 128
LAG1, LAG2 = 16, 32   # wavefront lags (multiples of RING)
RING = 16             # xz ring slots
HR = 16               # hT time-ring length

# column permutation: keras gate order [i|f|g|o] (512 each) ->
# kernel order: per 512-chunk n: [i_n | f_n | o_n | g_n] (128 each)
_PERM = np.concatenate([
    np.concatenate([
        np.arange(0 + 128 * n, 128 + 128 * n),        # i_n
        np.arange(512 + 128 * n, 640 + 128 * n),      # f_n
        np.arange(1536 + 128 * n, 1664 + 128 * n),    # o_n
        np.arange(1024 + 128 * n, 1152 + 128 * n),    # g_n
    ]) for n in range(4)
])
# gate col indices in permuted layout
_I_COLS = np.concatenate([np.arange(n * 512, n * 512 + 128) for n in range(4)])
_F_COLS = _I_COLS + 128
_O_COLS = _I_COLS + 256
_G_COLS = _I_COLS + 384


def _bn_fold(g, b, m, v):
    sc = g / np.sqrt(v + EPS)
    sh = b - m * sc
    return sc.astype(np.float32), sh.astype(np.float32)


def _bf16(a):
    import ml_dtypes
    return np.ascontiguousarray(np.asarray(a, np.float32).astype(
        ml_dtypes.bfloat16))


def prep_weights(inp):
    """Host-side constant folding. Returns dict of prepared arrays."""
    f = np.float32
    o = {}
    o['W0p'] = np.ascontiguousarray(inp['W0'][:, _PERM], f)
    o['b0row'] = np.ascontiguousarray(inp['b0'][_PERM], f)
    o['U0b'] = _bf16(inp['U0'][:, _PERM])
    o['U1b'] = _bf16(inp['U1'][:, _PERM])
    o['U2b'] = _bf16(inp['U2'][:, _PERM])
    sc0, sh0 = _bn_fold(inp['bn0_g'], inp['bn0_b'], inp['bn0_m'], inp['bn0_v'])
    o['W1b'] = _bf16((sc0[:, None] * inp['W1'])[:, _PERM])
    o['b1b'] = _bf16((inp['b1'] + sh0 @ inp['W1'])[_PERM])
    sc1, sh1 = _bn_fold(inp['bn1_g'], inp['bn1_b'], inp['bn1_m'], inp['bn1_v'])
    o['W2b'] = _bf16((sc1[:, None] * inp['W2'])[:, _PERM])
    o['b2b'] = _bf16((inp['b2'] + sh1 @ inp['W2'])[_PERM])
    o['Wab'] = _bf16(inp['Wa'])
    o['ba'] = np.ascontiguousarray(inp['ba'], f)
    # pooled = sum_t a*h2 (no 1/T); fold 1/T into Wd1
    o['Wd1p'] = np.ascontiguousarray(inp['Wd1'] / np.float32(T), f)
    o['bd1'] = np.ascontiguousarray(inp['bd1'], f)
    sc2, sh2 = _bn_fold(inp['bn2_g'], inp['bn2_b'], inp['bn2_m'], inp['bn2_v'])
    o['Wd2p'] = np.ascontiguousarray(sc2[:, None] * inp['Wd2'], f)
    o['bd2p'] = np.ascontiguousarray(inp['bd2'] + sh2 @ inp['Wd2'], f)
    o['Wd3'] = np.ascontiguousarray(inp['Wd3'], f)
    o['bd3'] = np.ascontiguousarray(inp['bd3'], f)
    # selector for summing rows (t,b) -> b : sel[p, b] = 1 if p % BL == b
    sel = np.zeros((P, BL), f)
    sel[np.arange(P), np.arange(P) % BL] = 1.0
    o['sel'] = sel
    o['ident'] = np.eye(P, dtype=f)
    eye80 = np.zeros((80, BL), f)
    for l in range(3):
        eye80[32 * l:32 * l + BL] = np.eye(BL, dtype=f)
    o['eye80b'] = _bf16(eye80)
    o['ones1'] = np.ones((1, P), f)
    o['ones32b'] = _bf16(np.ones((1, 32), f))
    return o


def _sigmoid(x):
    return 1.0 / (1.0 + np.exp(-x))


def numpy_forward(inp, t_steps=T, b_rows=B):
    """Numpy mirror of the kernel math (folded weights, permuted gates),
    in fp32 (no bf16 effects). Validates the host-side folds."""
    w = prep_weights(inp)
    x = np.asarray(inp['x'], np.float32)[:b_rows, :t_steps]
    U = {0: np.asarray(w['U0b'], np.float32),
         1: np.asarray(w['U1b'], np.float32),
         2: np.asarray(w['U2b'], np.float32)}
    W1 = np.asarray(w['W1b'], np.float32)
    W2 = np.asarray(w['W2b'], np.float32)

    def scan(xz, Um):
        bsz = xz.shape[0]
        h = np.zeros((bsz, H), np.float32)
        c = np.zeros((bsz, H), np.float32)
        hs = np.empty((bsz, t_steps, H), np.float32)
        for t in range(t_steps):
            z = xz[:, t] + h @ Um
            i = _sigmoid(z[:, _I_COLS]); f = _sigmoid(z[:, _F_COLS])
            o_ = _sigmoid(z[:, _O_COLS]); g = np.tanh(z[:, _G_COLS])
            c = f * c + i * g
            h = o_ * np.tanh(c)
            hs[:, t] = h
        return hs  # [B, T, H]

    xz0 = np.einsum('btd,dg->btg', x, w['W0p']) + w['b0row']
    h0 = scan(xz0, U[0])
    xz1 = np.einsum('bth,hg->btg', h0, W1) + np.asarray(w['b1b'], np.float32)
    h1 = scan(xz1, U[1])
    xz2 = np.einsum('bth,hg->btg', h1, W2) + np.asarray(w['b2b'], np.float32)
    h2 = scan(xz2, U[2])

    e = np.tanh(np.einsum('bth,hk->btk', h2, np.asarray(w['Wab'], np.float32))
                + w['ba'])
    s = e.sum(-1)
    s = s - s.max(axis=1, keepdims=True)
    a = np.exp(s); a = a / a.sum(axis=1, keepdims=True)
    pooled = np.einsum('bt,bth->bh', a, h2)
    d1 = np.maximum(pooled @ w['Wd1p'] + w['bd1'], 0)
    d2 = np.maximum(d1 @ w['Wd2p'] + w['bd2p'], 0)
    return d2 @ w['Wd3'] + w['bd3']


# ---------------------------------------------------------------------------
# Bass program
# ---------------------------------------------------------------------------

def build_nc(t_steps=T):
    import concourse.bacc as bacc
    import concourse.mybir as mybir
    import concourse.tile as tile
    from contextlib import ExitStack

    f32 = mybir.dt.float32
    f32r = mybir.dt.float32r
    bf16 = mybir.dt.bfloat16
    AF = mybir.ActivationFunctionType
    OP = mybir.AluOpType
    M = t_steps * BL
    MT = M // P
    TPB = P // BL  # timesteps per 128-row tile (8)
    NSLOT = t_steps + LAG2

    nc = bacc.Bacc("TRN2", target_bir_lowering=False, debug=False,
                   num_devices=NC)

    def din(name, shape, dt=f32):
        return nc.dram_tensor(name, list(shape), dt, kind="ExternalInput")

    x_d = din('xT', (D, t_steps, BL))
    W0p = din('W0p', (D, G4)); b0row = din('b0row', (G4,))
    U_d = [din('U0b', (H, G4), bf16), din('U1b', (H, G4), bf16),
           din('U2b', (H, G4), bf16)]
    W_d = {1: din('W1b', (H, G4), bf16), 2: din('W2b', (H, G4), bf16)}
    brow_d = {1: din('b1b', (G4,), bf16), 2: din('b2b', (G4,), bf16)}
    Wab = din('Wab', (H, H), bf16); ba = din('ba', (H,))
    Wd1p = din('Wd1p', (H, P)); bd1 = din('bd1', (P,))
    Wd2p = din('Wd2p', (P, 64)); bd2p = din('bd2p', (64,))
    Wd3 = din('Wd3', (64, 5)); bd3 = din('bd3', (5,))
    sel_d = din('sel', (P, BL))
    ident_d = din('ident', (P, P))
    eye80_d = din('eye80b', (80, BL), bf16)
    ones1_d = din('ones1', (1, P))
    ones32_d = din('ones32b', (1, 32), bf16)
    outT = nc.dram_tensor('outT', [5, BL], f32, kind="ExternalOutput")

    # DRAM temps
    xz_d = nc.dram_tensor('xz_buf', [M, G4], bf16)
    h2T = nc.dram_tensor('h2T', [4, P, t_steps, BL], bf16)
    h2rows = nc.dram_tensor('h2rows', [M, H], bf16)
    s_dram = nc.dram_tensor('s_dram', [M], f32)
    a_dram = nc.dram_tensor('a_dram', [M], f32)

    NSL = [slice(n * 512, (n + 1) * 512) for n in range(4)]
    ROWS = [slice(32 * l, 32 * l + BL) for l in range(3)]

    with tile.TileContext(nc) as tc:
        with ExitStack() as gctx:
            gconst = gctx.enter_context(tc.tile_pool(name="gconst", bufs=1))
            ident = gconst.tile([P, P], f32)
            nc.sync.dma_start(ident[:], ident_d[:, :])
            sel = gconst.tile([P, BL], f32)
            nc.sync.dma_start(sel[:], sel_d[:, :])

            # ---------------- layer-0 input projection pass ----------------
            def xz_pass():
                with ExitStack() as ctx:
                    cst = ctx.enter_context(tc.tile_pool(name="p0c", bufs=1))
                    W_stg = cst.tile([P, G4], f32, name="p0Ws")
                    nc.sync.dma_start(W_stg[:], W0p[:, :])
                    W_sb = cst.tile([P, G4], f32r, name="p0W")
                    nc.any.tensor_copy(W_sb[:], W_stg[:])
                    ones_s = cst.tile([1, P], f32, name="p0o_s")
                    nc.sync.dma_start(ones_s[:], ones1_d[:, :])
                    ones_sb = cst.tile([1, P], f32r, name="p0o")
                    nc.any.tensor_copy(ones_sb[:], ones_s[:])
                    b0_s = cst.tile([1, G4], f32, name="p0b_s")
                    nc.sync.dma_start(b0_s[:], b0row[None, :])
                    b0_sb = cst.tile([1, G4], f32r, name="p0b")
                    nc.any.tensor_copy(b0_sb[:], b0_s[:])
                    io = ctx.enter_context(tc.tile_pool(name="p0io", bufs=3))
                    ps = ctx.enter_context(
                        tc.tile_pool(name="p0ps", bufs=2, space="PSUM"))
                    for m in range(MT):
                        km_s = io.tile([P, P], f32, tag="km_s")
                        nc.sync.dma_start(
                            km_s[:].rearrange("p (t b) -> p t b", b=BL),
                            x_d[:, m * TPB:(m + 1) * TPB, :])
                        km = io.tile([P, P], f32r, tag="km")
                        nc.any.tensor_copy(km[:], km_s[:])
                        zp = ps.tile([P, G4], f32, tag="zp")
                        for n in range(4):
                            nc.tensor.matmul(zp[:, NSL[n]], ones_sb[:],
                                             b0_sb[:, NSL[n]],
                                             start=True, stop=False)
                            nc.tensor.matmul(zp[:, NSL[n]], km[:],
                                             W_sb[:, NSL[n]],
                                             start=False, stop=True)
                        ob = io.tile([P, G4], bf16, tag="ob")
                        nc.vector.tensor_copy(ob[:], zp[:])
                        nc.sync.dma_start(xz_d[m * P:(m + 1) * P, :], ob[:])

            # ---------------- 3-layer wavefront scan ----------------
            def wavefront():
                with ExitStack() as ctx:
                    cst = ctx.enter_context(tc.tile_pool(name="wfc", bufs=1))
                    U_sb = []
                    for l in range(3):
                        u = cst.tile([P, 4, G4], bf16, name=f"wfU{l}")
                        nc.sync.dma_start(
                            u[:], U_d[l].rearrange("(k p) n -> p k n", p=P))
                        U_sb.append(u)
                    W_sb = {}
                    for l in (1, 2):
                        w = cst.tile([P, 4, G4], bf16, name=f"wfW{l}")
                        nc.sync.dma_start(
                            w[:], W_d[l].rearrange("(k p) n -> p k n", p=P))
                        W_sb[l] = w
                    brow = {}
                    for l in (1, 2):
                        bt = cst.tile([1, G4], bf16, name=f"wfbr{l}")
                        nc.sync.dma_start(bt[:], brow_d[l][None, :])
                        brow[l] = bt
                    eye80 = cst.tile([80, BL], bf16, name="wfeye")
                    nc.sync.dma_start(eye80[:], eye80_d[:, :])
                    ones32 = cst.tile([1, 32], bf16, name="wfo32")
                    nc.sync.dma_start(ones32[:], ones32_d[:, :])
                    # persistent state
                    c_sb = cst.tile([80, 4, P], f32, name="wf_c")
                    nc.vector.memset(c_sb[:], 0.0)
                    cf_sb = cst.tile([80, 4, P], f32, name="wf_cf")
                    ig_sb = cst.tile([80, 4, P], bf16, name="wf_ig")
                    tch = cst.tile([80, 4, P], bf16, name="wf_tch")
                    sigb = cst.tile([80, 4, 384], bf16, name="wf_sig")
                    gb = cst.tile([80, 4, P], bf16, name="wf_g")
                    h_bf = cst.tile([96, H], bf16, name="wf_h")
                    nc.vector.memset(h_bf[:], 0.0)
                    tmp_bf = cst.tile([96, H], bf16, name="wf_tmp")
                    nc.vector.memset(tmp_bf[:], 0.0)
                    hT = [cst.tile([P, 4, HR, BL], bf16, name=f"wfhT{l}")
                          for l in range(3)]
                    for l in range(3):
                        nc.vector.memset(hT[l][:], 0.0)
                    ring = cst.tile([80, RING, G4], bf16, name="wf_ring")
                    nc.vector.memset(ring[:], 0.0)
                    psp = ctx.enter_context(
                        tc.tile_pool(name="wfps", bufs=1, space="PSUM"))
                    zfull = psp.tile([P, G4], f32, name="wf_zps")
                    nc.vector.memset(zfull[:], 0.0)
                    wk = ctx.enter_context(tc.tile_pool(name="wfwk", bufs=2))

                    def prefetch_xz0(t0):
                        # load xz rows for steps [t0, t0+4) into ring rows 0:16
                        if 0 <= t0 < t_steps:
                            nt = min(4, t_steps - t0)
                            nc.gpsimd.dma_start(
                                ring[0:BL, t0 % RING:t0 % RING + nt, :],
                                xz_d[t0 * BL:(t0 + nt) * BL, :]
                                .rearrange("(t b) g -> b t g", b=BL))

                    prefetch_xz0(0)

                    SCAT_ENG = [nc.sync, nc.scalar, nc.gpsimd]
                    pj_state = {}  # live pj tile: (L, tau, stage)

                    def emit_pj_chunks(L, tau, stage, ns):
                        """Emit pj matmuls + evict pieces for chunks ns."""
                        r0 = tau % HR
                        for n in ns:
                            nc.tensor.matmul(
                                zfull[96:128, NSL[n]], ones32[:],
                                brow[L][:, NSL[n]],
                                start=True, stop=False,
                                tile_position=(0, 96))
                        for k in range(4):
                            stat = hT[L - 1][:, k, r0:r0 + 2, :]
                            for n in ns:
                                nc.tensor.matmul(
                                    zfull[96:128, NSL[n]], stat,
                                    W_sb[L][:, k, NSL[n]],
                                    start=False, stop=(k == 3),
                                    tile_position=(0, 96))
                        for n in ns:
                            if n % 2 == 0:
                                nc.vector.tensor_copy(
                                    stage[:, NSL[n]], zfull[96:128, NSL[n]])
                            else:
                                nc.scalar.activation(
                                    stage[:, NSL[n]], zfull[96:128, NSL[n]],
                                    AF.Identity)

                    for s in range(NSLOT):
                        ts_ = [s, s - LAG1, s - LAG2]
                        act = [0 <= t < t_steps for t in ts_]
                        rec = [act[l] and ts_[l] >= 1 for l in range(3)]
                        r = s % RING
                        if s % 4 == 0:
                            prefetch_xz0(s + 4)

                        # ---- projection work on strip 3, 2-phase ----
                        # Per slot: finish one layer's 2-step tile (chunks
                        # 2-3 + ring DMAs) and start the other layer's
                        # (chunks 0-1). L1 tiles start on even slots
                        # (tau=s-4), L2 on odd slots (tau=s-17).
                        fin = 2 if s % 2 == 0 else 1
                        if fin in pj_state:
                            L, tau, stage = pj_state.pop(fin)
                            emit_pj_chunks(L, tau, stage, (2, 3))
                            r0 = tau % HR
                            for i2 in range(2):
                                nc.scalar.dma_start(
                                    ring[32 * L:32 * L + BL, r0 + i2, :],
                                    stage[BL * i2:BL * (i2 + 1), :])
                        beg = 1 if s % 2 == 0 else 2
                        tau = s - 4 if beg == 1 else s - 17
                        if 0 <= tau <= t_steps - 2:
                            stage = wk.tile([32, G4], bf16, tag="stage")
                            pj_state[beg] = (beg, tau, stage)
                            emit_pj_chunks(beg, tau, stage, (0, 1))

                        # ---- xz+bias inject (K=16 identity matmuls) ----
                        for n in range(4):
                            for l in range(3):
                                if act[l]:
                                    nc.tensor.matmul(
                                        zfull[ROWS[l], NSL[n]],
                                        eye80[32 * l:32 * l + BL, :],
                                        ring[32 * l:32 * l + BL, r, NSL[n]],
                                        start=True, stop=not rec[l],
                                        tile_position=(32 * l, 32 * l))

                        # ---- recurrent matmuls + per-chunk gates/chain ----
                        for n in range(4):
                            for k in range(4):
                                for l in range(3):
                                    if rec[l]:
                                        nc.tensor.matmul(
                                            zfull[ROWS[l], NSL[n]],
                                            hT[l][:, k, (ts_[l] - 1) % HR,
                                                  :],
                                            U_sb[l][:, k, NSL[n]],
                                            start=False, stop=(k == 3),
                                            tile_position=(0, 32 * l))
                            # gates + c/h chain for hidden quarter n
                            # (contiguous APs throughout)
                            nc.scalar.activation(
                                sigb[:, n, :],
                                zfull[0:80, 512 * n:512 * n + 384],
                                AF.Sigmoid)
                            nc.scalar.activation(
                                gb[:, n, :],
                                zfull[0:80, 512 * n + 384:512 * (n + 1)],
                                AF.Tanh)
                            nc.vector.tensor_tensor(
                                cf_sb[:, n, :], c_sb[:, n, :],
                                sigb[:, n, 128:256], OP.mult)
                            nc.vector.tensor_tensor(
                                ig_sb[:, n, :], sigb[:, n, 0:128],
                                gb[:, n, :], OP.mult)
                            nc.vector.tensor_tensor(
                                c_sb[:, n, :], cf_sb[:, n, :],
                                ig_sb[:, n, :], OP.add)
                            nc.scalar.activation(
                                tch[:, n, :], c_sb[:, n, :], AF.Tanh)
                            nc.vector.tensor_tensor(
                                h_bf[0:80, 128 * n:128 * (n + 1)],
                                sigb[:, n, 256:384], tch[:, n, :], OP.mult)

                        # ---- transpose h and scatter into hT rings ----
                        nc.vector.transpose(tmp_bf[:], h_bf[:])
                        for j in range(4):
                            for l in range(3):
                                if not act[l]:
                                    continue
                                SCAT_ENG[(j + l) % 3].dma_start(
                                    hT[l][32 * j:32 * j + 32, :,
                                          ts_[l] % HR, :],
                                    tmp_bf[32 * l:32 * l + 32, :]
                                    .rearrange("p (k c) -> p k c", c=P)
                                    [:, :, 32 * j:32 * j + BL])

                        # ---- layer-2 outputs for attention ----
                        if act[2]:
                            t2 = ts_[2]
                            nc.sync.dma_start(
                                h2rows[t2 * BL:(t2 + 1) * BL, :],
                                h_bf[64:64 + BL, :])
                            if t2 % 4 == 3:
                                r4 = (t2 - 3) % HR
                                nc.sync.dma_start(
                                    h2T.rearrange("k p t b -> p k (t b)")
                                    [:, :, (t2 - 3) * BL:(t2 + 1) * BL],
                                    hT[2][:, :, r4:r4 + 4, :]
                                    .rearrange("p k t b -> p k (t b)"))

            # ---------------- run pipeline ----------------
            xz_pass()
            wavefront()

            # ---------------- attention ----------------
            with ExitStack() as ctx:
                cst = ctx.enter_context(tc.tile_pool(name="atc", bufs=1))
                Wa_sb = cst.tile([P, 4, H], bf16, name="atWa")
                nc.sync.dma_start(
                    Wa_sb[:], Wab.rearrange("(k p) n -> p k n", p=P))
                ba_rep = cst.tile([P, H], f32)
                nc.sync.dma_start(ba_rep[:], ba[None, :].to_broadcast((P, H)))
                s_sb = cst.tile([P, MT], f32)
                io = ctx.enter_context(tc.tile_pool(name="atio", bufs=3))
                ps = ctx.enter_context(
                    tc.tile_pool(name="atps", bufs=2, space="PSUM"))
                # e-pass: s[(t,b)] = sum_k tanh(h2 @ Wa + ba)
                for m in range(MT):
                    kxm = io.tile([P, 4, TPB, BL], bf16, tag="kxm")
                    for k in range(4):
                        nc.sync.dma_start(
                            kxm[:, k],
                            h2T[k, :, m * TPB:(m + 1) * TPB, :])
                    ep = ps.tile([P, H], f32, tag="ep")
                    for k in range(4):
                        nc.tensor.matmul(
                            ep[:], kxm[:, k], Wa_sb[:, k, :],
                            start=(k == 0), stop=(k == 3))
                    e_sb = io.tile([P, H], f32, tag="e")
                    nc.vector.tensor_tensor(e_sb[:], ep[:], ba_rep[:], OP.add)
                    e_t = io.tile([P, H], f32, tag="et")
                    nc.scalar.activation(e_t[:], e_sb[:], AF.Tanh,
                                         accum_out=s_sb[:, m:m + 1])

                # s (row layout [P, MT]) -> sT [BL, t_steps] via flat DRAM
                nc.sync.dma_start(
                    s_dram.rearrange("(m p) -> p m", p=P), s_sb[:])
                sT = cst.tile([BL, t_steps], f32)
                nc.sync.dma_start(
                    sT[:], s_dram.rearrange("(t b) -> b t", b=BL))
                mx = cst.tile([BL, 1], f32)
                nc.vector.reduce_max(mx[:], sT[:], axis=mybir.AxisListType.X)
                nmx = cst.tile([BL, 1], f32)
                nc.vector.tensor_scalar_mul(nmx[:], mx[:], -1.0)
                ex = cst.tile([BL, t_steps], f32)
                sm = cst.tile([BL, 1], f32)
                nc.scalar.activation(ex[:], sT[:], AF.Exp, bias=nmx[:],
                                     accum_out=sm[:])
                rs = cst.tile([BL, 1], f32)
                nc.vector.reciprocal(rs[:], sm[:])
                aT = cst.tile([BL, t_steps], f32)
                nc.vector.tensor_scalar_mul(aT[:], ex[:], rs[:])
                nc.sync.dma_start(
                    a_dram.rearrange("(t b) -> b t", b=BL), aT[:])
                a_row = cst.tile([P, MT], f32)
                nc.sync.dma_start(
                    a_row[:], a_dram.rearrange("(m p) -> p m", p=P))

                # pooled[b, :] = sum_rows sel * (a * h2)   (f32 matmuls)
                pp = ctx.enter_context(
                    tc.tile_pool(name="atpp", bufs=1, space="PSUM"))
                ps1 = ctx.enter_context(
                    tc.tile_pool(name="atp1", bufs=1, space="PSUM"))
                pooled_ps = pp.tile([BL, H], f32)
                for m in range(MT):
                    h2t = io.tile([P, H], bf16, tag="h2t")
                    nc.sync.dma_start(h2t[:], h2rows[m * P:(m + 1) * P, :])
                    wrow = io.tile([P, H], f32, tag="wrow")
                    nc.vector.tensor_scalar_mul(wrow[:], h2t[:],
                                                a_row[:, m:m + 1])
                    nc.tensor.matmul(pooled_ps[:], sel[:], wrow[:],
                                     start=(m == 0), stop=(m == MT - 1))

                # pooledT via PE transpose
                pooled_sb = cst.tile([BL, H], f32)
                nc.vector.tensor_copy(pooled_sb[:], pooled_ps[:])
                ptp = ps1.tile([P, 4 * BL], f32, tag="ptp")
                for k in range(4):
                    nc.tensor.transpose(
                        ptp[:, k * BL:(k + 1) * BL],
                        pooled_sb[:, k * P:(k + 1) * P], ident[0:BL, 0:BL])
                pooledT = cst.tile([P, 4, BL], f32r)
                nc.vector.tensor_copy(
                    pooledT[:], ptp[:].rearrange("p (k b) -> p k b", k=4))

                # ---------------- dense head ----------------
                def load_r(pool, dram_ap, shape, name):
                    stg = pool.tile(shape, f32, name=name + "_stg")
                    nc.sync.dma_start(stg[:], dram_ap)
                    t_ = pool.tile(shape, f32r, name=name)
                    nc.any.tensor_copy(t_[:], stg[:])
                    return t_

                Wd1_sb = load_r(cst, Wd1p.rearrange("(k p) n -> p k n", p=P),
                                [P, 4, P], "hWd1")
                bd1_sb = cst.tile([P, 1], f32)
                nc.sync.dma_start(bd1_sb[:], bd1[:, None])
                Wd2_sb = load_r(cst, Wd2p[:, :], [P, 64], "hWd2")
                bd2_sb = cst.tile([64, 1], f32)
                nc.sync.dma_start(bd2_sb[:], bd2p[:, None])
                Wd3_sb = load_r(cst, Wd3[:, :], [64, 5], "hWd3")
                bd3_sb = cst.tile([5, 1], f32)
                nc.sync.dma_start(bd3_sb[:], bd3[:, None])

                d1p = ps1.tile([P, BL], f32, tag="d1p")
                for k in range(4):
                    nc.tensor.matmul(d1p[:], Wd1_sb[:, k, :], pooledT[:, k, :],
                                     start=(k == 0), stop=(k == 3))
                d1 = cst.tile([P, BL], f32r)
                nc.scalar.activation(d1[:], d1p[:], AF.Relu, bias=bd1_sb[:])
                d2p = ps1.tile([64, BL], f32, tag="d2p")
                nc.tensor.matmul(d2p[:], Wd2_sb[:], d1[:], start=True,
                                 stop=True)
                d2 = cst.tile([64, BL], f32r)
                nc.scalar.activation(d2[:], d2p[:], AF.Relu, bias=bd2_sb[:])
                d3p = ps1.tile([5, BL], f32, tag="d3p")
                nc.tensor.matmul(d3p[:], Wd3_sb[:], d2[:], start=True,
                                 stop=True)
                d3 = cst.tile([5, BL], f32)
                nc.scalar.activation(d3[:], d3p[:], AF.Identity, bias=bd3_sb[:])
                nc.sync.dma_start(outT[:, :], d3[:])

    nc.compile()
    return nc


@functools.lru_cache(maxsize=2)
def _compiled(t_steps):
    return build_nc(t_steps)


def _make_in_maps(inputs):
    w = prep_weights(inputs)
    x = np.ascontiguousarray(np.asarray(inputs['x'], np.float32))
    base = {k: w[k] for k in (
        'W0p', 'b0row', 'U0b', 'U1b', 'U2b', 'W1b', 'b1b', 'W2b', 'b2b',
        'Wab', 'ba', 'Wd1p', 'bd1', 'Wd2p', 'bd2p', 'Wd3', 'bd3', 'sel',
        'ident', 'eye80b', 'ones1', 'ones32b')}
    in_maps = []
    for c in range(NC):
        m = dict(base)
        m['xT'] = np.ascontiguousarray(
            x[c * BL:(c + 1) * BL].transpose(2, 1, 0))
        in_maps.append(m)
    return in_maps


def kernel(**inputs):
    from concourse import bass_utils
    nc = _compiled(T)
    in_maps = _make_in_maps(inputs)
    res = bass_utils.run_bass_kernel_spmd(nc, in_maps, core_ids=list(range(NC)))
    out = np.concatenate([np.asarray(res.results[c]['outT']).T
                          for c in range(NC)], axis=0)
    return np.ascontiguousarray(out, np.float32)


def timed_run(tmpdir=None, **inputs):
    """Run with NTFF profiling; returns BassKernelResults."""
    from concourse import bass_utils
    nc = _compiled(T)
    in_maps = _make_in_maps(inputs)
    res = bass_utils.run_bass_kernel_spmd(
        nc, in_maps, core_ids=list(range(NC)), trace=True, tmpdir=tmpdir)
    return res



# revision 7
# speedup vs baseline: 1.2748x; 1.2748x over previous
"""Trainium2 Bass kernel for stacked-LSTM + attention + dense head (v2).

Model (per reference):
  3x LSTM(H=512, return_sequences) with inference BatchNorm between layers,
  attention pooling over time, then Dense(128)+BN+Dense(64)+Dense(5).
  B=128, T=512, D=128, H=512, fp32.

Strategy: data-parallel over batch (16 rows/core on 8 cores). Per core a
3-layer wavefront with lags 16/32 (ring-aligned). Key structure vs v1:
  - xz (input projection) is INJECTED into PSUM by K=16 identity matmuls
    (plus the recurrent matmuls accumulate on top), removing the wide DVE
    adds from the critical chain.
  - Biases enter via K=1 ones-row matmuls (no DVE bias adds anywhere).
  - Gates use an interleaved column permutation [i_n|f_n|o_n|g_n] per
    512-chunk so sigmoid/tanh run per-chunk, hidden under the MM stream.
  - Projections for layers 1/2 run on PE column strip 3 (2 steps/tile),
    evicted by a single tensor_copy and DMA'd into a 16-slot SBUF ring.
  - The c/h elementwise chain is split across DVE and GpSimd.
  - h transpose via DVE StreamTranspose + 12 small scatter DMAs spread
    over 4 issuing engines.

Self-contained: hardcodes shapes; no reads of reference.py/spec.json.
"""

import functools

import numpy as np

B, T, D, H = 128, 512, 128, 512
NC = 8
BL = B // NC          # batch rows per core
G4 = 4 * H            # gate width 2048
EPS = 1e-3
P = 128
LAG1, LAG2 = 16, 32   # wavefront lags (multiples of RING)
RING = 16             # xz ring slots
HR = 16               # hT time-ring length

# column permutation: keras gate order [i|f|g|o] (512 each) ->
# kernel order: per 512-chunk n: [i_n | f_n | o_n | g_n] (128 each)
_PERM = np.concatenate([
    np.concatenate([
        np.arange(0 + 128 * n, 128 + 128 * n),        # i_n
        np.arange(512 + 128 * n, 640 + 128 * n),      # f_n
        np.arange(1536 + 128 * n, 1664 + 128 * n),    # o_n
        np.arange(1024 + 128 * n, 1152 + 128 * n),    # g_n
    ]) for n in range(4)
])
# gate col indices in permuted layout
_I_COLS = np.concatenate([np.arange(n * 512, n * 512 + 128) for n in range(4)])
_F_COLS = _I_COLS + 128
_O_COLS = _I_COLS + 256
_G_COLS = _I_COLS + 384


def _bn_fold(g, b, m, v):
    sc = g / np.sqrt(v + EPS)
    sh = b - m * sc
    return sc.astype(np.float32), sh.astype(np.float32)


def _bf16(a):
    import ml_dtypes
    return np.ascontiguousarray(np.asarray(a, np.float32).astype(
        ml_dtypes.bfloat16))


def prep_weights(inp):
    """Host-side constant folding. Returns dict of prepared arrays."""
    f = np.float32
    o = {}
    o['W0p'] = np.ascontiguousarray(inp['W0'][:, _PERM], f)
    o['b0row'] = np.ascontiguousarray(inp['b0'][_PERM], f)
    o['U0b'] = _bf16(inp['U0'][:, _PERM])
    o['U1b'] = _bf16(inp['U1'][:, _PERM])
    o['U2b'] = _bf16(inp['U2'][:, _PERM])
    sc0, sh0 = _bn_fold(inp['bn0_g'], inp['bn0_b'], inp['bn0_m'], inp['bn0_v'])
    o['W1b'] = _bf16((sc0[:, None] * inp['W1'])[:, _PERM])
    o['b1b'] = _bf16((inp['b1'] + sh0 @ inp['W1'])[_PERM])
    sc1, sh1 = _bn_fold(inp['bn1_g'], inp['bn1_b'], inp['bn1_m'], inp['bn1_v'])
    o['W2b'] = _bf16((sc1[:, None] * inp['W2'])[:, _PERM])
    o['b2b'] = _bf16((inp['b2'] + sh1 @ inp['W2'])[_PERM])
    o['Wab'] = _bf16(inp['Wa'])
    o['ba'] = np.ascontiguousarray(inp['ba'], f)
    # pooled = sum_t a*h2 (no 1/T); fold 1/T into Wd1
    o['Wd1p'] = np.ascontiguousarray(inp['Wd1'] / np.float32(T), f)
    o['bd1'] = np.ascontiguousarray(inp['bd1'], f)
    sc2, sh2 = _bn_fold(inp['bn2_g'], inp['bn2_b'], inp['bn2_m'], inp['bn2_v'])
    o['Wd2p'] = np.ascontiguousarray(sc2[:, None] * inp['Wd2'], f)
    o['bd2p'] = np.ascontiguousarray(inp['bd2'] + sh2 @ inp['Wd2'], f)
    o['Wd3'] = np.ascontiguousarray(inp['Wd3'], f)
    o['bd3'] = np.ascontiguousarray(inp['bd3'], f)
    # selector for summing rows (t,b) -> b : sel[p, b] = 1 if p % BL == b
    sel = np.zeros((P, BL), f)
    sel[np.arange(P), np.arange(P) % BL] = 1.0
    o['sel'] = sel
    o['ident'] = np.eye(P, dtype=f)
    eye80sq = np.zeros((80, 80), f)
    for l in range(3):
        sl = slice(32 * l, 32 * l + BL)
        eye80sq[sl, sl] = np.eye(BL, dtype=f)
    o['eye80sq'] = _bf16(eye80sq)
    o['ones1'] = np.ones((1, P), f)
    o['ones32b'] = _bf16(np.ones((1, 32), f))
    return o


def _sigmoid(x):
    return 1.0 / (1.0 + np.exp(-x))


def numpy_forward(inp, t_steps=T, b_rows=B):
    """Numpy mirror of the kernel math (folded weights, permuted gates),
    in fp32 (no bf16 effects). Validates the host-side folds."""
    w = prep_weights(inp)
    x = np.asarray(inp['x'], np.float32)[:b_rows, :t_steps]
    U = {0: np.asarray(w['U0b'], np.float32),
         1: np.asarray(w['U1b'], np.float32),
         2: np.asarray(w['U2b'], np.float32)}
    W1 = np.asarray(w['W1b'], np.float32)
    W2 = np.asarray(w['W2b'], np.float32)

    def scan(xz, Um):
        bsz = xz.shape[0]
        h = np.zeros((bsz, H), np.float32)
        c = np.zeros((bsz, H), np.float32)
        hs = np.empty((bsz, t_steps, H), np.float32)
        for t in range(t_steps):
            z = xz[:, t] + h @ Um
            i = _sigmoid(z[:, _I_COLS]); f = _sigmoid(z[:, _F_COLS])
            o_ = _sigmoid(z[:, _O_COLS]); g = np.tanh(z[:, _G_COLS])
            c = f * c + i * g
            h = o_ * np.tanh(c)
            hs[:, t] = h
        return hs  # [B, T, H]

    xz0 = np.einsum('btd,dg->btg', x, w['W0p']) + w['b0row']
    h0 = scan(xz0, U[0])
    xz1 = np.einsum('bth,hg->btg', h0, W1) + np.asarray(w['b1b'], np.float32)
    h1 = scan(xz1, U[1])
    xz2 = np.einsum('bth,hg->btg', h1, W2) + np.asarray(w['b2b'], np.float32)
    h2 = scan(xz2, U[2])

    e = np.tanh(np.einsum('bth,hk->btk', h2, np.asarray(w['Wab'], np.float32))
                + w['ba'])
    s = e.sum(-1)
    s = s - s.max(axis=1, keepdims=True)
    a = np.exp(s); a = a / a.sum(axis=1, keepdims=True)
    pooled = np.einsum('bt,bth->bh', a, h2)
    d1 = np.maximum(pooled @ w['Wd1p'] + w['bd1'], 0)
    d2 = np.maximum(d1 @ w['Wd2p'] + w['bd2p'], 0)
    return d2 @ w['Wd3'] + w['bd3']


# ---------------------------------------------------------------------------
# Bass program
# ---------------------------------------------------------------------------

def build_nc(t_steps=T):
    import concourse.bacc as bacc
    import concourse.mybir as mybir
    import concourse.tile as tile
    from contextlib import ExitStack

    f32 = mybir.dt.float32
    f32r = mybir.dt.float32r
    bf16 = mybir.dt.bfloat16
    AF = mybir.ActivationFunctionType
    OP = mybir.AluOpType
    M = t_steps * BL
    MT = M // P
    TPB = P // BL  # timesteps per 128-row tile (8)
    NSLOT = t_steps + LAG2

    nc = bacc.Bacc("TRN2", target_bir_lowering=False, debug=False,
                   num_devices=NC)

    def din(name, shape, dt=f32):
        return nc.dram_tensor(name, list(shape), dt, kind="ExternalInput")

    x_d = din('xT', (D, t_steps, BL))
    W0p = din('W0p', (D, G4)); b0row = din('b0row', (G4,))
    U_d = [din('U0b', (H, G4), bf16), din('U1b', (H, G4), bf16),
           din('U2b', (H, G4), bf16)]
    W_d = {1: din('W1b', (H, G4), bf16), 2: din('W2b', (H, G4), bf16)}
    brow_d = {1: din('b1b', (G4,), bf16), 2: din('b2b', (G4,), bf16)}
    Wab = din('Wab', (H, H), bf16); ba = din('ba', (H,))
    Wd1p = din('Wd1p', (H, P)); bd1 = din('bd1', (P,))
    Wd2p = din('Wd2p', (P, 64)); bd2p = din('bd2p', (64,))
    Wd3 = din('Wd3', (64, 5)); bd3 = din('bd3', (5,))
    sel_d = din('sel', (P, BL))
    ident_d = din('ident', (P, P))
    eye80sq_d = din('eye80sq', (80, 80), bf16)
    ones1_d = din('ones1', (1, P))
    ones32_d = din('ones32b', (1, 32), bf16)
    outT = nc.dram_tensor('outT', [5, BL], f32, kind="ExternalOutput")

    # DRAM temps
    xz_d = nc.dram_tensor('xz_buf', [M, G4], bf16)
    h2T = nc.dram_tensor('h2T', [4, P, t_steps, BL], bf16)
    h2rows = nc.dram_tensor('h2rows', [M, H], bf16)
    s_dram = nc.dram_tensor('s_dram', [M], f32)
    a_dram = nc.dram_tensor('a_dram', [M], f32)

    NSL = [slice(n * 512, (n + 1) * 512) for n in range(4)]
    ROWS = [slice(32 * l, 32 * l + BL) for l in range(3)]

    with tile.TileContext(nc) as tc:
        with ExitStack() as gctx:
            gconst = gctx.enter_context(tc.tile_pool(name="gconst", bufs=1))
            ident = gconst.tile([P, P], f32)
            nc.sync.dma_start(ident[:], ident_d[:, :])
            sel = gconst.tile([P, BL], f32)
            nc.sync.dma_start(sel[:], sel_d[:, :])

            # ---------------- layer-0 input projection pass ----------------
            def xz_pass():
                with ExitStack() as ctx:
                    cst = ctx.enter_context(tc.tile_pool(name="p0c", bufs=1))
                    W_stg = cst.tile([P, G4], f32, name="p0Ws")
                    nc.sync.dma_start(W_stg[:], W0p[:, :])
                    W_sb = cst.tile([P, G4], f32r, name="p0W")
                    nc.any.tensor_copy(W_sb[:], W_stg[:])
                    ones_s = cst.tile([1, P], f32, name="p0o_s")
                    nc.sync.dma_start(ones_s[:], ones1_d[:, :])
                    ones_sb = cst.tile([1, P], f32r, name="p0o")
                    nc.any.tensor_copy(ones_sb[:], ones_s[:])
                    b0_s = cst.tile([1, G4], f32, name="p0b_s")
                    nc.sync.dma_start(b0_s[:], b0row[None, :])
                    b0_sb = cst.tile([1, G4], f32r, name="p0b")
                    nc.any.tensor_copy(b0_sb[:], b0_s[:])
                    io = ctx.enter_context(tc.tile_pool(name="p0io", bufs=3))
                    ps = ctx.enter_context(
                        tc.tile_pool(name="p0ps", bufs=2, space="PSUM"))
                    for m in range(MT):
                        km_s = io.tile([P, P], f32, tag="km_s")
                        nc.sync.dma_start(
                            km_s[:].rearrange("p (t b) -> p t b", b=BL),
                            x_d[:, m * TPB:(m + 1) * TPB, :])
                        km = io.tile([P, P], f32r, tag="km")
                        nc.any.tensor_copy(km[:], km_s[:])
                        zp = ps.tile([P, G4], f32, tag="zp")
                        for n in range(4):
                            nc.tensor.matmul(zp[:, NSL[n]], ones_sb[:],
                                             b0_sb[:, NSL[n]],
                                             start=True, stop=False)
                            nc.tensor.matmul(zp[:, NSL[n]], km[:],
                                             W_sb[:, NSL[n]],
                                             start=False, stop=True)
                        ob = io.tile([P, G4], bf16, tag="ob")
                        nc.vector.tensor_copy(ob[:], zp[:])
                        nc.sync.dma_start(xz_d[m * P:(m + 1) * P, :], ob[:])

            # ---------------- 3-layer wavefront scan ----------------
            def wavefront():
                with ExitStack() as ctx:
                    cst = ctx.enter_context(tc.tile_pool(name="wfc", bufs=1))
                    U_sb = []
                    for l in range(3):
                        u = cst.tile([P, 4, G4], bf16, name=f"wfU{l}")
                        nc.sync.dma_start(
                            u[:], U_d[l].rearrange("(k p) n -> p k n", p=P))
                        U_sb.append(u)
                    W_sb = {}
                    for l in (1, 2):
                        w = cst.tile([P, 4, G4], bf16, name=f"wfW{l}")
                        nc.sync.dma_start(
                            w[:], W_d[l].rearrange("(k p) n -> p k n", p=P))
                        W_sb[l] = w
                    brow = {}
                    for l in (1, 2):
                        bt = cst.tile([1, G4], bf16, name=f"wfbr{l}")
                        nc.sync.dma_start(bt[:], brow_d[l][None, :])
                        brow[l] = bt
                    eye80 = cst.tile([80, 80], bf16, name="wfeye")
                    nc.sync.dma_start(eye80[:], eye80sq_d[:, :])
                    ones32 = cst.tile([1, 32], bf16, name="wfo32")
                    nc.sync.dma_start(ones32[:], ones32_d[:, :])
                    # persistent state
                    c_sb = cst.tile([80, 4, P], f32, name="wf_c")
                    nc.vector.memset(c_sb[:], 0.0)
                    cf_sb = cst.tile([80, 4, P], f32, name="wf_cf")
                    ig_sb = cst.tile([80, 4, P], bf16, name="wf_ig")
                    tch = cst.tile([80, 4, P], bf16, name="wf_tch")
                    sigb = cst.tile([80, 4, 384], bf16, name="wf_sig")
                    gb = cst.tile([80, 4, P], bf16, name="wf_g")
                    h_bf = cst.tile([96, H], bf16, name="wf_h")
                    nc.vector.memset(h_bf[:], 0.0)
                    tmp_bf = cst.tile([96, H], bf16, name="wf_tmp")
                    nc.vector.memset(tmp_bf[:], 0.0)
                    hT = [cst.tile([P, 4, HR, BL], bf16, name=f"wfhT{l}")
                          for l in range(3)]
                    for l in range(3):
                        nc.vector.memset(hT[l][:], 0.0)
                    ring = cst.tile([80, RING, G4], bf16, name="wf_ring")
                    nc.vector.memset(ring[:], 0.0)
                    psp = ctx.enter_context(
                        tc.tile_pool(name="wfps", bufs=1, space="PSUM"))
                    zA = psp.tile([P, G4], f32, name="wf_zA")
                    nc.vector.memset(zA[:], 0.0)
                    zB = psp.tile([P, G4], f32, name="wf_zB")
                    nc.vector.memset(zB[:], 0.0)
                    Zpp = [zA, zB]
                    wk = ctx.enter_context(tc.tile_pool(name="wfwk", bufs=3))

                    def prefetch_xz0(t0):
                        # load xz rows for steps [t0, t0+4) into ring rows 0:16
                        if 0 <= t0 < t_steps:
                            nt = min(4, t_steps - t0)
                            nc.gpsimd.dma_start(
                                ring[0:BL, t0 % RING:t0 % RING + nt, :],
                                xz_d[t0 * BL:(t0 + nt) * BL, :]
                                .rearrange("(t b) g -> b t g", b=BL))

                    prefetch_xz0(0)

                    SCAT_ENG = [nc.sync, nc.scalar, nc.gpsimd]
                    pj_state = {}  # live pj tile: (L, tau, stage)

                    def emit_pj_chunks(L, tau, stage, ns, zpj):
                        """Emit pj matmuls + evict pieces for chunks ns."""
                        r0 = tau % HR
                        for n in ns:
                            nc.tensor.matmul(
                                zpj[96:128, NSL[n]], ones32[:],
                                brow[L][:, NSL[n]],
                                start=True, stop=False,
                                tile_position=(0, 96))
                        for k in range(4):
                            stat = hT[L - 1][:, k, r0:r0 + 2, :]
                            for n in ns:
                                nc.tensor.matmul(
                                    zpj[96:128, NSL[n]], stat,
                                    W_sb[L][:, k, NSL[n]],
                                    start=False, stop=(k == 3),
                                    tile_position=(0, 96))
                        for n in ns:
                            if n % 2 == 0:
                                nc.vector.tensor_copy(
                                    stage[:, NSL[n]], zpj[96:128, NSL[n]])
                            else:
                                nc.scalar.activation(
                                    stage[:, NSL[n]], zpj[96:128, NSL[n]],
                                    AF.Identity)

                    for s in range(NSLOT):
                        ts_ = [s, s - LAG1, s - LAG2]
                        act = [0 <= t < t_steps for t in ts_]
                        rec = [act[l] and ts_[l] >= 1 for l in range(3)]
                        any_rec = any(rec)
                        r = s % RING
                        z = Zpp[s % 2]        # this slot's gate banks
                        zpj = Zpp[(s + 1) % 2]  # pj writes opposite tile
                        if s % 4 == 0:
                            prefetch_xz0(s + 4)

                        # ---- projection work on strip 3, 2-phase ----
                        # Per slot: finish one layer's 2-step tile (chunks
                        # 2-3 + ring DMAs) and start the other layer's
                        # (chunks 0-1). L1 tiles start on even slots
                        # (tau=s-4), L2 on odd slots (tau=s-17).
                        fin = 2 if s % 2 == 0 else 1
                        if fin in pj_state:
                            L, tau, stage = pj_state.pop(fin)
                            emit_pj_chunks(L, tau, stage, (2, 3), zpj)
                            r0 = tau % HR
                            for i2 in range(2):
                                nc.scalar.dma_start(
                                    ring[32 * L:32 * L + BL, r0 + i2, :],
                                    stage[BL * i2:BL * (i2 + 1), :])
                        beg = 1 if s % 2 == 0 else 2
                        tau = s - 4 if beg == 1 else s - 17
                        if 0 <= tau <= t_steps - 2:
                            stage = wk.tile([32, G4], bf16, tag="stage")
                            pj_state[beg] = (beg, tau, stage)
                            emit_pj_chunks(beg, tau, stage, (0, 1), zpj)

                        # ---- per-chunk tail: tanh(c), h, transpose ----
                        def tail(n):
                            nc.scalar.activation(
                                tch[:, n, :], c_sb[:, n, :], AF.Tanh)
                            nc.vector.tensor_tensor(
                                h_bf[0:80, 128 * n:128 * (n + 1)],
                                sigb[:, n, 256:384], tch[:, n, :], OP.mult)
                            nc.vector.transpose(
                                tmp_bf[:, 128 * n:128 * (n + 1)],
                                h_bf[:, 128 * n:128 * (n + 1)])

                        # ---- per-chunk: inject + recurrent MMs + gates ----
                        for n in range(4):
                            nc.tensor.matmul(
                                z[0:80, NSL[n]], eye80[:],
                                ring[0:80, r, NSL[n]],
                                start=True, stop=not any_rec,
                                skip_group_check=True)
                            for k in range(4):
                                for l in range(3):
                                    if rec[l]:
                                        nc.tensor.matmul(
                                            z[ROWS[l], NSL[n]],
                                            hT[l][:, k, (ts_[l] - 1) % HR,
                                                  :],
                                            U_sb[l][:, k, NSL[n]],
                                            start=False, stop=(k == 3),
                                            tile_position=(0, 32 * l),
                                            skip_group_check=True)
                            # gates for hidden quarter n
                            nc.scalar.activation(
                                sigb[:, n, :],
                                z[0:80, 512 * n:512 * n + 384],
                                AF.Sigmoid)
                            nc.scalar.activation(
                                gb[:, n, :],
                                z[0:80, 512 * n + 384:512 * (n + 1)],
                                AF.Tanh)
                            nc.vector.tensor_tensor(
                                cf_sb[:, n, :], c_sb[:, n, :],
                                sigb[:, n, 128:256], OP.mult)
                            nc.gpsimd.tensor_tensor(
                                ig_sb[:, n, :], sigb[:, n, 0:128],
                                gb[:, n, :], OP.mult)
                            nc.vector.tensor_tensor(
                                c_sb[:, n, :], cf_sb[:, n, :],
                                ig_sb[:, n, :], OP.add)
                            if n >= 1:
                                tail(n - 1)
                        tail(3)

                        # ---- scatter transposed h into hT rings ----
                        for j in range(4):
                            for l in range(3):
                                if not act[l]:
                                    continue
                                SCAT_ENG[(j + l) % 3].dma_start(
                                    hT[l][32 * j:32 * j + 32, :,
                                          ts_[l] % HR, :],
                                    tmp_bf[32 * l:32 * l + 32, :]
                                    .rearrange("p (k c) -> p k c", c=P)
                                    [:, :, 32 * j:32 * j + BL])

                        # ---- layer-2 outputs for attention ----
                        if act[2]:
                            t2 = ts_[2]
                            nc.sync.dma_start(
                                h2rows[t2 * BL:(t2 + 1) * BL, :],
                                h_bf[64:64 + BL, :])
                            if t2 % 4 == 3:
                                r4 = (t2 - 3) % HR
                                nc.sync.dma_start(
                                    h2T.rearrange("k p t b -> p k (t b)")
                                    [:, :, (t2 - 3) * BL:(t2 + 1) * BL],
                                    hT[2][:, :, r4:r4 + 4, :]
                                    .rearrange("p k t b -> p k (t b)"))

            # ---------------- run pipeline ----------------
            xz_pass()
            wavefront()

            # ---------------- attention ----------------
            with ExitStack() as ctx:
                cst = ctx.enter_context(tc.tile_pool(name="atc", bufs=1))
                Wa_sb = cst.tile([P, 4, H], bf16, name="atWa")
                nc.sync.dma_start(
                    Wa_sb[:], Wab.rearrange("(k p) n -> p k n", p=P))
                ba_rep = cst.tile([P, H], f32)
                nc.sync.dma_start(ba_rep[:], ba[None, :].to_broadcast((P, H)))
                s_sb = cst.tile([P, MT], f32)
                io = ctx.enter_context(tc.tile_pool(name="atio", bufs=3))
                ps = ctx.enter_context(
                    tc.tile_pool(name="atps", bufs=2, space="PSUM"))
                # e-pass: s[(t,b)] = sum_k tanh(h2 @ Wa + ba)
                for m in range(MT):
                    kxm = io.tile([P, 4, TPB, BL], bf16, tag="kxm")
                    for k in range(4):
                        nc.sync.dma_start(
                            kxm[:, k],
                            h2T[k, :, m * TPB:(m + 1) * TPB, :])
                    ep = ps.tile([P, H], f32, tag="ep")
                    for k in range(4):
                        nc.tensor.matmul(
                            ep[:], kxm[:, k], Wa_sb[:, k, :],
                            start=(k == 0), stop=(k == 3))
                    e_sb = io.tile([P, H], f32, tag="e")
                    nc.vector.tensor_tensor(e_sb[:], ep[:], ba_rep[:], OP.add)
                    e_t = io.tile([P, H], f32, tag="et")
                    nc.scalar.activation(e_t[:], e_sb[:], AF.Tanh,
                                         accum_out=s_sb[:, m:m + 1])

                # s (row layout [P, MT]) -> sT [BL, t_steps] via flat DRAM
                nc.sync.dma_start(
                    s_dram.rearrange("(m p) -> p m", p=P), s_sb[:])
                sT = cst.tile([BL, t_steps], f32)
                nc.sync.dma_start(
                    sT[:], s_dram.rearrange("(t b) -> b t", b=BL))
                mx = cst.tile([BL, 1], f32)
                nc.vector.reduce_max(mx[:], sT[:], axis=mybir.AxisListType.X)
                nmx = cst.tile([BL, 1], f32)
                nc.vector.tensor_scalar_mul(nmx[:], mx[:], -1.0)
                ex = cst.tile([BL, t_steps], f32)
                sm = cst.tile([BL, 1], f32)
                nc.scalar.activation(ex[:], sT[:], AF.Exp, bias=nmx[:],
                                     accum_out=sm[:])
                rs = cst.tile([BL, 1], f32)
                nc.vector.reciprocal(rs[:], sm[:])
                aT = cst.tile([BL, t_steps], f32)
                nc.vector.tensor_scalar_mul(aT[:], ex[:], rs[:])
                nc.sync.dma_start(
                    a_dram.rearrange("(t b) -> b t", b=BL), aT[:])
                a_row = cst.tile([P, MT], f32)
                nc.sync.dma_start(
                    a_row[:], a_dram.rearrange("(m p) -> p m", p=P))

                # pooled[b, :] = sum_rows sel * (a * h2)   (f32 matmuls)
                pp = ctx.enter_context(
                    tc.tile_pool(name="atpp", bufs=1, space="PSUM"))
                ps1 = ctx.enter_context(
                    tc.tile_pool(name="atp1", bufs=1, space="PSUM"))
                pooled_ps = pp.tile([BL, H], f32)
                for m in range(MT):
                    h2t = io.tile([P, H], bf16, tag="h2t")
                    nc.sync.dma_start(h2t[:], h2rows[m * P:(m + 1) * P, :])
                    wrow = io.tile([P, H], f32, tag="wrow")
                    nc.vector.tensor_scalar_mul(wrow[:], h2t[:],
                                                a_row[:, m:m + 1])
                    nc.tensor.matmul(pooled_ps[:], sel[:], wrow[:],
                                     start=(m == 0), stop=(m == MT - 1))

                # pooledT via PE transpose
                pooled_sb = cst.tile([BL, H], f32)
                nc.vector.tensor_copy(pooled_sb[:], pooled_ps[:])
                ptp = ps1.tile([P, 4 * BL], f32, tag="ptp")
                for k in range(4):
                    nc.tensor.transpose(
                        ptp[:, k * BL:(k + 1) * BL],
                        pooled_sb[:, k * P:(k + 1) * P], ident[0:BL, 0:BL])
                pooledT = cst.tile([P, 4, BL], f32r)
                nc.vector.tensor_copy(
                    pooledT[:], ptp[:].rearrange("p (k b) -> p k b", k=4))

                # ---------------- dense head ----------------
                def load_r(pool, dram_ap, shape, name):
                    stg = pool.tile(shape, f32, name=name + "_stg")
                    nc.sync.dma_start(stg[:], dram_ap)
                    t_ = pool.tile(shape, f32r, name=name)
                    nc.any.tensor_copy(t_[:], stg[:])
                    return t_

                Wd1_sb = load_r(cst, Wd1p.rearrange("(k p) n -> p k n", p=P),
                                [P, 4, P], "hWd1")
                bd1_sb = cst.tile([P, 1], f32)
                nc.sync.dma_start(bd1_sb[:], bd1[:, None])
                Wd2_sb = load_r(cst, Wd2p[:, :], [P, 64], "hWd2")
                bd2_sb = cst.tile([64, 1], f32)
                nc.sync.dma_start(bd2_sb[:], bd2p[:, None])
                Wd3_sb = load_r(cst, Wd3[:, :], [64, 5], "hWd3")
                bd3_sb = cst.tile([5, 1], f32)
                nc.sync.dma_start(bd3_sb[:], bd3[:, None])

                d1p = ps1.tile([P, BL], f32, tag="d1p")
                for k in range(4):
                    nc.tensor.matmul(d1p[:], Wd1_sb[:, k, :], pooledT[:, k, :],
                                     start=(k == 0), stop=(k == 3))
                d1 = cst.tile([P, BL], f32r)
                nc.scalar.activation(d1[:], d1p[:], AF.Relu, bias=bd1_sb[:])
                d2p = ps1.tile([64, BL], f32, tag="d2p")
                nc.tensor.matmul(d2p[:], Wd2_sb[:], d1[:], start=True,
                                 stop=True)
                d2 = cst.tile([64, BL], f32r)
                nc.scalar.activation(d2[:], d2p[:], AF.Relu, bias=bd2_sb[:])
                d3p = ps1.tile([5, BL], f32, tag="d3p")
                nc.tensor.matmul(d3p[:], Wd3_sb[:], d2[:], start=True,
                                 stop=True)
                d3 = cst.tile([5, BL], f32)
                nc.scalar.activation(d3[:], d3p[:], AF.Identity, bias=bd3_sb[:])
                nc.sync.dma_start(outT[:, :], d3[:])

    nc.compile()
    return nc


@functools.lru_cache(maxsize=2)
def _compiled(t_steps):
    return build_nc(t_steps)


def _make_in_maps(inputs):
    w = prep_weights(inputs)
    x = np.ascontiguousarray(np.asarray(inputs['x'], np.float32))
    base = {k: w[k] for k in (
        'W0p', 'b0row', 'U0b', 'U1b', 'U2b', 'W1b', 'b1b', 'W2b', 'b2b',
        'Wab', 'ba', 'Wd1p', 'bd1', 'Wd2p', 'bd2p', 'Wd3', 'bd3', 'sel',
        'ident', 'eye80sq', 'ones1', 'ones32b')}
    in_maps = []
    for c in range(NC):
        m = dict(base)
        m['xT'] = np.ascontiguousarray(
            x[c * BL:(c + 1) * BL].transpose(2, 1, 0))
        in_maps.append(m)
    return in_maps


def kernel(**inputs):
    from concourse import bass_utils
    nc = _compiled(T)
    in_maps = _make_in_maps(inputs)
    res = bass_utils.run_bass_kernel_spmd(nc, in_maps, core_ids=list(range(NC)))
    out = np.concatenate([np.asarray(res.results[c]['outT']).T
                          for c in range(NC)], axis=0)
    return np.ascontiguousarray(out, np.float32)


def timed_run(tmpdir=None, **inputs):
    """Run with NTFF profiling; returns BassKernelResults."""
    from concourse import bass_utils
    nc = _compiled(T)
    in_maps = _make_in_maps(inputs)
    res = bass_utils.run_bass_kernel_spmd(
        nc, in_maps, core_ids=list(range(NC)), trace=True, tmpdir=tmpdir)
    return res



# revision 13
# speedup vs baseline: 1.5631x; 1.2262x over previous
"""Trainium2 Bass kernel for stacked-LSTM + attention + dense head (v2).

Model (per reference):
  3x LSTM(H=512, return_sequences) with inference BatchNorm between layers,
  attention pooling over time, then Dense(128)+BN+Dense(64)+Dense(5).
  B=128, T=512, D=128, H=512, fp32.

Strategy: data-parallel over batch (16 rows/core on 8 cores). Per core a
3-layer wavefront with lags 16/32 (ring-aligned). Key structure vs v1:
  - xz (input projection) is INJECTED into PSUM by K=16 identity matmuls
    (plus the recurrent matmuls accumulate on top), removing the wide DVE
    adds from the critical chain.
  - Biases enter via K=1 ones-row matmuls (no DVE bias adds anywhere).
  - Gates use an interleaved column permutation [i_n|f_n|o_n|g_n] per
    512-chunk so sigmoid/tanh run per-chunk, hidden under the MM stream.
  - Projections for layers 1/2 run on PE column strip 3 (2 steps/tile),
    evicted by a single tensor_copy and DMA'd into a 16-slot SBUF ring.
  - The c/h elementwise chain is split across DVE and GpSimd.
  - h transpose via DVE StreamTranspose + 12 small scatter DMAs spread
    over 4 issuing engines.

Self-contained: hardcodes shapes; no reads of reference.py/spec.json.
"""

import functools

import numpy as np

B, T, D, H = 128, 512, 128, 512
NC = 8
BL = B // NC          # batch rows per core
G4 = 4 * H            # gate width 2048
EPS = 1e-3
P = 128
LAG1, LAG2 = 16, 32   # wavefront lags (multiples of RING)
RING = 16             # xz ring slots
HR = 16               # hT time-ring length

# column permutation: keras gate order [i|f|g|o] (512 each) ->
# kernel order: per 512-chunk n: [i_n | f_n | o_n | g_n] (128 each)
_PERM = np.concatenate([
    np.concatenate([
        np.arange(0 + 128 * n, 128 + 128 * n),        # i_n
        np.arange(512 + 128 * n, 640 + 128 * n),      # f_n
        np.arange(1536 + 128 * n, 1664 + 128 * n),    # o_n
        np.arange(1024 + 128 * n, 1152 + 128 * n),    # g_n
    ]) for n in range(4)
])
# gate col indices in permuted layout
_I_COLS = np.concatenate([np.arange(n * 512, n * 512 + 128) for n in range(4)])
_F_COLS = _I_COLS + 128
_O_COLS = _I_COLS + 256
_G_COLS = _I_COLS + 384


def _bn_fold(g, b, m, v):
    sc = g / np.sqrt(v + EPS)
    sh = b - m * sc
    return sc.astype(np.float32), sh.astype(np.float32)


def _bf16(a):
    import ml_dtypes
    return np.ascontiguousarray(np.asarray(a, np.float32).astype(
        ml_dtypes.bfloat16))


def prep_weights(inp):
    """Host-side constant folding. Returns dict of prepared arrays."""
    f = np.float32
    o = {}
    o['W0p'] = np.ascontiguousarray(inp['W0'][:, _PERM], f)
    o['b0row'] = np.ascontiguousarray(inp['b0'][_PERM], f)
    o['U0b'] = _bf16(inp['U0'][:, _PERM])
    o['U1b'] = _bf16(inp['U1'][:, _PERM])
    o['U2b'] = _bf16(inp['U2'][:, _PERM])
    sc0, sh0 = _bn_fold(inp['bn0_g'], inp['bn0_b'], inp['bn0_m'], inp['bn0_v'])
    o['W1b'] = _bf16((sc0[:, None] * inp['W1'])[:, _PERM])
    o['b1b'] = _bf16((inp['b1'] + sh0 @ inp['W1'])[_PERM])
    sc1, sh1 = _bn_fold(inp['bn1_g'], inp['bn1_b'], inp['bn1_m'], inp['bn1_v'])
    o['W2b'] = _bf16((sc1[:, None] * inp['W2'])[:, _PERM])
    o['b2b'] = _bf16((inp['b2'] + sh1 @ inp['W2'])[_PERM])
    o['Wab'] = _bf16(inp['Wa'])
    o['ba'] = np.ascontiguousarray(inp['ba'], f)
    # pooled = sum_t a*h2 (no 1/T); fold 1/T into Wd1
    o['Wd1p'] = np.ascontiguousarray(inp['Wd1'] / np.float32(T), f)
    o['bd1'] = np.ascontiguousarray(inp['bd1'], f)
    sc2, sh2 = _bn_fold(inp['bn2_g'], inp['bn2_b'], inp['bn2_m'], inp['bn2_v'])
    o['Wd2p'] = np.ascontiguousarray(sc2[:, None] * inp['Wd2'], f)
    o['bd2p'] = np.ascontiguousarray(inp['bd2'] + sh2 @ inp['Wd2'], f)
    o['Wd3'] = np.ascontiguousarray(inp['Wd3'], f)
    o['bd3'] = np.ascontiguousarray(inp['bd3'], f)
    # selector for summing rows (t,b) -> b : sel[p, b] = 1 if p % BL == b
    sel = np.zeros((P, BL), f)
    sel[np.arange(P), np.arange(P) % BL] = 1.0
    o['sel'] = sel
    o['ident'] = np.eye(P, dtype=f)
    eye80sq = np.zeros((80, 80), f)
    for l in range(3):
        sl = slice(32 * l, 32 * l + BL)
        eye80sq[sl, sl] = np.eye(BL, dtype=f)
    o['eye80sq'] = _bf16(eye80sq)
    o['ones1'] = np.ones((1, P), f)
    o['ones32b'] = _bf16(np.ones((1, 32), f))
    return o


def _sigmoid(x):
    return 1.0 / (1.0 + np.exp(-x))


def numpy_forward(inp, t_steps=T, b_rows=B):
    """Numpy mirror of the kernel math (folded weights, permuted gates),
    in fp32 (no bf16 effects). Validates the host-side folds."""
    w = prep_weights(inp)
    x = np.asarray(inp['x'], np.float32)[:b_rows, :t_steps]
    U = {0: np.asarray(w['U0b'], np.float32),
         1: np.asarray(w['U1b'], np.float32),
         2: np.asarray(w['U2b'], np.float32)}
    W1 = np.asarray(w['W1b'], np.float32)
    W2 = np.asarray(w['W2b'], np.float32)

    def scan(xz, Um):
        bsz = xz.shape[0]
        h = np.zeros((bsz, H), np.float32)
        c = np.zeros((bsz, H), np.float32)
        hs = np.empty((bsz, t_steps, H), np.float32)
        for t in range(t_steps):
            z = xz[:, t] + h @ Um
            i = _sigmoid(z[:, _I_COLS]); f = _sigmoid(z[:, _F_COLS])
            o_ = _sigmoid(z[:, _O_COLS]); g = np.tanh(z[:, _G_COLS])
            c = f * c + i * g
            h = o_ * np.tanh(c)
            hs[:, t] = h
        return hs  # [B, T, H]

    xz0 = np.einsum('btd,dg->btg', x, w['W0p']) + w['b0row']
    h0 = scan(xz0, U[0])
    xz1 = np.einsum('bth,hg->btg', h0, W1) + np.asarray(w['b1b'], np.float32)
    h1 = scan(xz1, U[1])
    xz2 = np.einsum('bth,hg->btg', h1, W2) + np.asarray(w['b2b'], np.float32)
    h2 = scan(xz2, U[2])

    e = np.tanh(np.einsum('bth,hk->btk', h2, np.asarray(w['Wab'], np.float32))
                + w['ba'])
    s = e.sum(-1)
    s = s - s.max(axis=1, keepdims=True)
    a = np.exp(s); a = a / a.sum(axis=1, keepdims=True)
    pooled = np.einsum('bt,bth->bh', a, h2)
    d1 = np.maximum(pooled @ w['Wd1p'] + w['bd1'], 0)
    d2 = np.maximum(d1 @ w['Wd2p'] + w['bd2p'], 0)
    return d2 @ w['Wd3'] + w['bd3']


# ---------------------------------------------------------------------------
# Bass program
# ---------------------------------------------------------------------------

def build_nc(t_steps=T):
    import concourse.bacc as bacc
    import concourse.mybir as mybir
    import concourse.tile as tile
    from contextlib import ExitStack

    f32 = mybir.dt.float32
    f32r = mybir.dt.float32r
    bf16 = mybir.dt.bfloat16
    AF = mybir.ActivationFunctionType
    OP = mybir.AluOpType
    M = t_steps * BL
    MT = M // P
    TPB = P // BL  # timesteps per 128-row tile (8)
    NSLOT = t_steps + LAG2

    nc = bacc.Bacc("TRN2", target_bir_lowering=False, debug=False,
                   num_devices=NC)

    def din(name, shape, dt=f32):
        return nc.dram_tensor(name, list(shape), dt, kind="ExternalInput")

    x_d = din('xT', (D, t_steps, BL))
    W0p = din('W0p', (D, G4)); b0row = din('b0row', (G4,))
    U_d = [din('U0b', (H, G4), bf16), din('U1b', (H, G4), bf16),
           din('U2b', (H, G4), bf16)]
    W_d = {1: din('W1b', (H, G4), bf16), 2: din('W2b', (H, G4), bf16)}
    brow_d = {1: din('b1b', (G4,), bf16), 2: din('b2b', (G4,), bf16)}
    Wab = din('Wab', (H, H), bf16); ba = din('ba', (H,))
    Wd1p = din('Wd1p', (H, P)); bd1 = din('bd1', (P,))
    Wd2p = din('Wd2p', (P, 64)); bd2p = din('bd2p', (64,))
    Wd3 = din('Wd3', (64, 5)); bd3 = din('bd3', (5,))
    sel_d = din('sel', (P, BL))
    ident_d = din('ident', (P, P))
    eye80sq_d = din('eye80sq', (80, 80), bf16)
    ones1_d = din('ones1', (1, P))
    ones32_d = din('ones32b', (1, 32), bf16)
    outT = nc.dram_tensor('outT', [5, BL], f32, kind="ExternalOutput")

    # DRAM temps
    xz_d = nc.dram_tensor('xz_buf', [M, G4], bf16)
    h2T = nc.dram_tensor('h2T', [4, P, t_steps, BL], bf16)
    h2rows = nc.dram_tensor('h2rows', [M, H], bf16)
    s_dram = nc.dram_tensor('s_dram', [M], f32)
    a_dram = nc.dram_tensor('a_dram', [M], f32)

    NSL = [slice(n * 512, (n + 1) * 512) for n in range(4)]
    ROWS = [slice(32 * l, 32 * l + BL) for l in range(3)]

    with tile.TileContext(nc) as tc:
        with ExitStack() as gctx:
            gconst = gctx.enter_context(tc.tile_pool(name="gconst", bufs=1))
            ident = gconst.tile([P, P], f32)
            nc.sync.dma_start(ident[:], ident_d[:, :])
            sel = gconst.tile([P, BL], f32)
            nc.sync.dma_start(sel[:], sel_d[:, :])

            # ---------------- layer-0 input projection pass ----------------
            def xz_pass():
                with ExitStack() as ctx:
                    cst = ctx.enter_context(tc.tile_pool(name="p0c", bufs=1))
                    W_stg = cst.tile([P, G4], f32, name="p0Ws")
                    nc.sync.dma_start(W_stg[:], W0p[:, :])
                    W_sb = cst.tile([P, G4], f32r, name="p0W")
                    nc.any.tensor_copy(W_sb[:], W_stg[:])
                    ones_s = cst.tile([1, P], f32, name="p0o_s")
                    nc.sync.dma_start(ones_s[:], ones1_d[:, :])
                    ones_sb = cst.tile([1, P], f32r, name="p0o")
                    nc.any.tensor_copy(ones_sb[:], ones_s[:])
                    b0_s = cst.tile([1, G4], f32, name="p0b_s")
                    nc.sync.dma_start(b0_s[:], b0row[None, :])
                    b0_sb = cst.tile([1, G4], f32r, name="p0b")
                    nc.any.tensor_copy(b0_sb[:], b0_s[:])
                    io = ctx.enter_context(tc.tile_pool(name="p0io", bufs=3))
                    ps = ctx.enter_context(
                        tc.tile_pool(name="p0ps", bufs=2, space="PSUM"))
                    for m in range(MT):
                        km_s = io.tile([P, P], f32, tag="km_s")
                        nc.sync.dma_start(
                            km_s[:].rearrange("p (t b) -> p t b", b=BL),
                            x_d[:, m * TPB:(m + 1) * TPB, :])
                        km = io.tile([P, P], f32r, tag="km")
                        nc.any.tensor_copy(km[:], km_s[:])
                        zp = ps.tile([P, G4], f32, tag="zp")
                        for n in range(4):
                            nc.tensor.matmul(zp[:, NSL[n]], ones_sb[:],
                                             b0_sb[:, NSL[n]],
                                             start=True, stop=False)
                            nc.tensor.matmul(zp[:, NSL[n]], km[:],
                                             W_sb[:, NSL[n]],
                                             start=False, stop=True)
                        ob = io.tile([P, G4], bf16, tag="ob")
                        nc.vector.tensor_copy(ob[:], zp[:])
                        nc.sync.dma_start(xz_d[m * P:(m + 1) * P, :], ob[:])

            # ---------------- 3-layer wavefront scan ----------------
            def wavefront():
                with ExitStack() as ctx:
                    cst = ctx.enter_context(tc.tile_pool(name="wfc", bufs=1))
                    U_sb = []
                    for l in range(3):
                        u = cst.tile([P, 4, G4], bf16, name=f"wfU{l}")
                        nc.sync.dma_start(
                            u[:], U_d[l].rearrange("(k p) n -> p k n", p=P))
                        U_sb.append(u)
                    W_sb = {}
                    for l in (1, 2):
                        w = cst.tile([P, 4, G4], bf16, name=f"wfW{l}")
                        nc.sync.dma_start(
                            w[:], W_d[l].rearrange("(k p) n -> p k n", p=P))
                        W_sb[l] = w
                    brow = {}
                    for l in (1, 2):
                        bt = cst.tile([1, G4], bf16, name=f"wfbr{l}")
                        nc.sync.dma_start(bt[:], brow_d[l][None, :])
                        brow[l] = bt
                    eye80 = cst.tile([80, 80], bf16, name="wfeye")
                    nc.sync.dma_start(eye80[:], eye80sq_d[:, :])
                    ones32 = cst.tile([1, 32], bf16, name="wfo32")
                    nc.sync.dma_start(ones32[:], ones32_d[:, :])
                    # persistent state: c split per chunk-half (tile-granular
                    # dep tracking — separate tiles avoid false WAR/RAW)
                    c_h = []
                    for hh in range(2):
                        t_ = cst.tile([80, 2, P], f32, name=f"wf_c{hh}")
                        nc.vector.memset(t_[:], 0.0)
                        c_h.append(t_)
                    # double-buffered (slot parity) per-half gate temps
                    cf_h = [[cst.tile([80, 2, P], f32, name=f"wf_cf{p}{hh}")
                             for hh in range(2)] for p in range(2)]
                    ig_h = [[cst.tile([80, 2, P], bf16, name=f"wf_ig{p}{hh}")
                             for hh in range(2)] for p in range(2)]
                    tch_h = [[cst.tile([80, 2, P], bf16, name=f"wf_tc{p}{hh}")
                              for hh in range(2)] for p in range(2)]
                    sig_h = [[cst.tile([80, 2, 384], bf16,
                                       name=f"wf_sg{p}{hh}")
                              for hh in range(2)] for p in range(2)]
                    gb_h = [[cst.tile([80, 2, P], bf16, name=f"wf_g{p}{hh}")
                             for hh in range(2)] for p in range(2)]
                    h_pp = []
                    tmp_pp = []
                    for p in range(2):
                        hb = cst.tile([96, H], bf16, name=f"wf_h{p}")
                        nc.vector.memset(hb[:], 0.0)
                        h_pp.append(hb)
                        tb = cst.tile([96, H], bf16, name=f"wf_tmp{p}")
                        nc.vector.memset(tb[:], 0.0)
                        tmp_pp.append(tb)
                    hT = [cst.tile([P, 4, HR, BL], bf16, name=f"wfhT{l}")
                          for l in range(3)]
                    for l in range(3):
                        nc.vector.memset(hT[l][:], 0.0)
                    ring = cst.tile([80, RING, G4], bf16, name="wf_ring")
                    nc.vector.memset(ring[:], 0.0)
                    psp = ctx.enter_context(
                        tc.tile_pool(name="wfps", bufs=1, space="PSUM"))
                    # one PSUM tile per (parity, chunk-pair): 4 x 2 banks
                    Zpp = []
                    for p in range(2):
                        row = []
                        for hh in range(2):
                            t_ = psp.tile([P, 1024], f32,
                                          name=f"wf_z{p}{hh}")
                            nc.vector.memset(t_[:], 0.0)
                            row.append(t_)
                        Zpp.append(row)
                    wk = ctx.enter_context(tc.tile_pool(name="wfwk", bufs=3))

                    def prefetch_xz0(t0):
                        # load xz rows for steps [t0, t0+4) into ring rows 0:16
                        if 0 <= t0 < t_steps:
                            nt = min(4, t_steps - t0)
                            nc.gpsimd.dma_start(
                                ring[0:BL, t0 % RING:t0 % RING + nt, :],
                                xz_d[t0 * BL:(t0 + nt) * BL, :]
                                .rearrange("(t b) g -> b t g", b=BL))

                    prefetch_xz0(0)

                    SCAT_ENG = [nc.sync, nc.gpsimd]
                    pj_state = {}  # live pj tile: (L, tau, stage)

                    def emit_pj_half(L, tau, stage, hh, zpj):
                        """Emit pj matmuls + evict for chunk pair hh."""
                        r0 = tau % HR
                        for q in range(2):
                            n = 2 * hh + q
                            nc.tensor.matmul(
                                zpj[hh][96:128, 512 * q:512 * (q + 1)],
                                ones32[:], brow[L][:, NSL[n]],
                                start=True, stop=False,
                                tile_position=(0, 96))
                        for k in range(4):
                            stat = hT[L - 1][:, k, r0:r0 + 2, :]
                            for q in range(2):
                                n = 2 * hh + q
                                nc.tensor.matmul(
                                    zpj[hh][96:128, 512 * q:512 * (q + 1)],
                                    stat, W_sb[L][:, k, NSL[n]],
                                    start=False, stop=(k == 3),
                                    tile_position=(0, 96))
                        nc.vector.tensor_copy(
                            stage[:, 1024 * hh:1024 * (hh + 1)],
                            zpj[hh][96:128, :])

                    for s in range(NSLOT):
                        ts_ = [s, s - LAG1, s - LAG2]
                        act = [0 <= t < t_steps for t in ts_]
                        rec = [act[l] and ts_[l] >= 1 for l in range(3)]
                        any_rec = any(rec)
                        r = s % RING
                        par = s % 2
                        z = Zpp[par]          # this slot's gate banks
                        zpj = Zpp[1 - par]    # pj writes opposite tiles
                        sigb = sig_h[par]; gb = gb_h[par]; tch = tch_h[par]
                        cfp = cf_h[par]; igp = ig_h[par]
                        h_bf = h_pp[par]; tmp_bf = tmp_pp[par]

                        # ---- per-half tail: tanh(c), h, transpose ----
                        def tail(hh):
                            nc.scalar.activation(
                                tch[hh][:], c_h[hh][:], AF.Tanh)
                            nc.vector.tensor_tensor(
                                h_bf[0:80, 256 * hh:256 * (hh + 1)]
                                .rearrange("p (q c) -> p q c", q=2),
                                sigb[hh][:, :, 256:384], tch[hh][:],
                                OP.mult)
                            nc.vector.transpose(
                                tmp_bf[:, 256 * hh:256 * (hh + 1)],
                                h_bf[:, 256 * hh:256 * (hh + 1)])

                        # ---- per-half: inject + recurrent MMs + gates ----
                        for hh in range(2):
                            for q in range(2):
                                n = 2 * hh + q
                                nc.tensor.matmul(
                                    z[hh][0:80, 512 * q:512 * (q + 1)],
                                    eye80[:], ring[0:80, r, NSL[n]],
                                    start=True, stop=not any_rec,
                                    skip_group_check=True)
                                for k in range(4):
                                    for l in range(3):
                                        if rec[l]:
                                            nc.tensor.matmul(
                                                z[hh][32 * l:32 * l + BL,
                                                      512 * q:512 * (q + 1)],
                                                hT[l][:, k,
                                                      (ts_[l] - 1) % HR, :],
                                                U_sb[l][:, k, NSL[n]],
                                                start=False, stop=(k == 3),
                                                tile_position=(0, 32 * l),
                                                skip_group_check=True)
                            # gates for chunk pair hh
                            nc.scalar.activation(
                                sigb[hh][:],
                                z[hh][0:80, :]
                                .rearrange("p (q c) -> p q c", q=2)
                                [:, :, 0:384],
                                AF.Sigmoid)
                            nc.scalar.activation(
                                gb[hh][:],
                                z[hh][0:80, :]
                                .rearrange("p (q c) -> p q c", q=2)
                                [:, :, 384:512],
                                AF.Tanh)
                            nc.vector.tensor_tensor(
                                cfp[hh][:], c_h[hh][:],
                                sigb[hh][:, :, 128:256], OP.mult)
                            nc.gpsimd.tensor_tensor(
                                igp[hh][:], sigb[hh][:, :, 0:128],
                                gb[hh][:], OP.mult)
                            nc.vector.tensor_tensor(
                                c_h[hh][:], cfp[hh][:], igp[hh][:], OP.add)
                            if hh == 1:
                                tail(0)
                        tail(1)

                        # ---- scatter transposed h into hT rings ----
                        for j in range(4):
                            for l in range(3):
                                if not act[l]:
                                    continue
                                SCAT_ENG[(j + l) % 2].dma_start(
                                    hT[l][32 * j:32 * j + 32, :,
                                          ts_[l] % HR, :],
                                    tmp_bf[32 * l:32 * l + 32, :]
                                    .rearrange("p (k c) -> p k c", c=P)
                                    [:, :, 32 * j:32 * j + BL])

                        # ---- layer-2 outputs for attention ----
                        if act[2]:
                            t2 = ts_[2]
                            nc.sync.dma_start(
                                h2rows[t2 * BL:(t2 + 1) * BL, :],
                                h_bf[64:64 + BL, :])
                            if t2 % 4 == 3:
                                r4 = (t2 - 3) % HR
                                nc.sync.dma_start(
                                    h2T.rearrange("k p t b -> p k (t b)")
                                    [:, :, (t2 - 3) * BL:(t2 + 1) * BL],
                                    hT[2][:, :, r4:r4 + 4, :]
                                    .rearrange("p k t b -> p k (t b)"))

                        # ---- projection work on strip 3, 2-phase ----
                        # Emitted at slot END so pj MMs fill the PE while
                        # this slot's gate chain drains, and pj evictions
                        # (DVE) queue after the gate ops.
                        fin = 2 if s % 2 == 0 else 1
                        if fin in pj_state:
                            L, tau, stage = pj_state.pop(fin)
                            emit_pj_half(L, tau, stage, 1, zpj)
                            r0 = tau % HR
                            for i2 in range(2):
                                nc.scalar.dma_start(
                                    ring[32 * L:32 * L + BL, r0 + i2, :],
                                    stage[BL * i2:BL * (i2 + 1), :])
                        beg = 1 if s % 2 == 0 else 2
                        tau = s - 4 if beg == 1 else s - 17
                        if 0 <= tau <= t_steps - 2:
                            stage = wk.tile([32, G4], bf16, tag="stage")
                            pj_state[beg] = (beg, tau, stage)
                            emit_pj_half(beg, tau, stage, 0, zpj)
                        if s % 4 == 0:
                            prefetch_xz0(s + 4)

            # ---------------- run pipeline ----------------
            xz_pass()
            wavefront()

            # ---------------- attention ----------------
            with ExitStack() as ctx:
                cst = ctx.enter_context(tc.tile_pool(name="atc", bufs=1))
                Wa_sb = cst.tile([P, 4, H], bf16, name="atWa")
                nc.sync.dma_start(
                    Wa_sb[:], Wab.rearrange("(k p) n -> p k n", p=P))
                ba_rep = cst.tile([P, H], f32)
                nc.sync.dma_start(ba_rep[:], ba[None, :].to_broadcast((P, H)))
                s_sb = cst.tile([P, MT], f32)
                io = ctx.enter_context(tc.tile_pool(name="atio", bufs=3))
                ps = ctx.enter_context(
                    tc.tile_pool(name="atps", bufs=2, space="PSUM"))
                # e-pass: s[(t,b)] = sum_k tanh(h2 @ Wa + ba)
                for m in range(MT):
                    kxm = io.tile([P, 4, TPB, BL], bf16, tag="kxm")
                    for k in range(4):
                        nc.sync.dma_start(
                            kxm[:, k],
                            h2T[k, :, m * TPB:(m + 1) * TPB, :])
                    ep = ps.tile([P, H], f32, tag="ep")
                    for k in range(4):
                        nc.tensor.matmul(
                            ep[:], kxm[:, k], Wa_sb[:, k, :],
                            start=(k == 0), stop=(k == 3))
                    e_sb = io.tile([P, H], f32, tag="e")
                    nc.vector.tensor_tensor(e_sb[:], ep[:], ba_rep[:], OP.add)
                    e_t = io.tile([P, H], f32, tag="et")
                    nc.scalar.activation(e_t[:], e_sb[:], AF.Tanh,
                                         accum_out=s_sb[:, m:m + 1])

                # s (row layout [P, MT]) -> sT [BL, t_steps] via flat DRAM
                nc.sync.dma_start(
                    s_dram.rearrange("(m p) -> p m", p=P), s_sb[:])
                sT = cst.tile([BL, t_steps], f32)
                nc.sync.dma_start(
                    sT[:], s_dram.rearrange("(t b) -> b t", b=BL))
                mx = cst.tile([BL, 1], f32)
                nc.vector.reduce_max(mx[:], sT[:], axis=mybir.AxisListType.X)
                nmx = cst.tile([BL, 1], f32)
                nc.vector.tensor_scalar_mul(nmx[:], mx[:], -1.0)
                ex = cst.tile([BL, t_steps], f32)
                sm = cst.tile([BL, 1], f32)
                nc.scalar.activation(ex[:], sT[:], AF.Exp, bias=nmx[:],
                                     accum_out=sm[:])
                rs = cst.tile([BL, 1], f32)
                nc.vector.reciprocal(rs[:], sm[:])
                aT = cst.tile([BL, t_steps], f32)
                nc.vector.tensor_scalar_mul(aT[:], ex[:], rs[:])
                nc.sync.dma_start(
                    a_dram.rearrange("(t b) -> b t", b=BL), aT[:])
                a_row = cst.tile([P, MT], f32)
                nc.sync.dma_start(
                    a_row[:], a_dram.rearrange("(m p) -> p m", p=P))

                # pooled[b, :] = sum_rows sel * (a * h2)   (f32 matmuls)
                pp = ctx.enter_context(
                    tc.tile_pool(name="atpp", bufs=1, space="PSUM"))
                ps1 = ctx.enter_context(
                    tc.tile_pool(name="atp1", bufs=1, space="PSUM"))
                pooled_ps = pp.tile([BL, H], f32)
                for m in range(MT):
                    h2t = io.tile([P, H], bf16, tag="h2t")
                    nc.sync.dma_start(h2t[:], h2rows[m * P:(m + 1) * P, :])
                    wrow = io.tile([P, H], f32, tag="wrow")
                    nc.vector.tensor_scalar_mul(wrow[:], h2t[:],
                                                a_row[:, m:m + 1])
                    nc.tensor.matmul(pooled_ps[:], sel[:], wrow[:],
                                     start=(m == 0), stop=(m == MT - 1))

                # pooledT via PE transpose
                pooled_sb = cst.tile([BL, H], f32)
                nc.vector.tensor_copy(pooled_sb[:], pooled_ps[:])
                ptp = ps1.tile([P, 4 * BL], f32, tag="ptp")
                for k in range(4):
                    nc.tensor.transpose(
                        ptp[:, k * BL:(k + 1) * BL],
                        pooled_sb[:, k * P:(k + 1) * P], ident[0:BL, 0:BL])
                pooledT = cst.tile([P, 4, BL], f32r)
                nc.vector.tensor_copy(
                    pooledT[:], ptp[:].rearrange("p (k b) -> p k b", k=4))

                # ---------------- dense head ----------------
                def load_r(pool, dram_ap, shape, name):
                    stg = pool.tile(shape, f32, name=name + "_stg")
                    nc.sync.dma_start(stg[:], dram_ap)
                    t_ = pool.tile(shape, f32r, name=name)
                    nc.any.tensor_copy(t_[:], stg[:])
                    return t_

                Wd1_sb = load_r(cst, Wd1p.rearrange("(k p) n -> p k n", p=P),
                                [P, 4, P], "hWd1")
                bd1_sb = cst.tile([P, 1], f32)
                nc.sync.dma_start(bd1_sb[:], bd1[:, None])
                Wd2_sb = load_r(cst, Wd2p[:, :], [P, 64], "hWd2")
                bd2_sb = cst.tile([64, 1], f32)
                nc.sync.dma_start(bd2_sb[:], bd2p[:, None])
                Wd3_sb = load_r(cst, Wd3[:, :], [64, 5], "hWd3")
                bd3_sb = cst.tile([5, 1], f32)
                nc.sync.dma_start(bd3_sb[:], bd3[:, None])

                d1p = ps1.tile([P, BL], f32, tag="d1p")
                for k in range(4):
                    nc.tensor.matmul(d1p[:], Wd1_sb[:, k, :], pooledT[:, k, :],
                                     start=(k == 0), stop=(k == 3))
                d1 = cst.tile([P, BL], f32r)
                nc.scalar.activation(d1[:], d1p[:], AF.Relu, bias=bd1_sb[:])
                d2p = ps1.tile([64, BL], f32, tag="d2p")
                nc.tensor.matmul(d2p[:], Wd2_sb[:], d1[:], start=True,
                                 stop=True)
                d2 = cst.tile([64, BL], f32r)
                nc.scalar.activation(d2[:], d2p[:], AF.Relu, bias=bd2_sb[:])
                d3p = ps1.tile([5, BL], f32, tag="d3p")
                nc.tensor.matmul(d3p[:], Wd3_sb[:], d2[:], start=True,
                                 stop=True)
                d3 = cst.tile([5, BL], f32)
                nc.scalar.activation(d3[:], d3p[:], AF.Identity, bias=bd3_sb[:])
                nc.sync.dma_start(outT[:, :], d3[:])

    nc.compile()
    return nc


@functools.lru_cache(maxsize=2)
def _compiled(t_steps):
    return build_nc(t_steps)


def _make_in_maps(inputs):
    w = prep_weights(inputs)
    x = np.ascontiguousarray(np.asarray(inputs['x'], np.float32))
    base = {k: w[k] for k in (
        'W0p', 'b0row', 'U0b', 'U1b', 'U2b', 'W1b', 'b1b', 'W2b', 'b2b',
        'Wab', 'ba', 'Wd1p', 'bd1', 'Wd2p', 'bd2p', 'Wd3', 'bd3', 'sel',
        'ident', 'eye80sq', 'ones1', 'ones32b')}
    in_maps = []
    for c in range(NC):
        m = dict(base)
        m['xT'] = np.ascontiguousarray(
            x[c * BL:(c + 1) * BL].transpose(2, 1, 0))
        in_maps.append(m)
    return in_maps


def kernel(**inputs):
    from concourse import bass_utils
    nc = _compiled(T)
    in_maps = _make_in_maps(inputs)
    res = bass_utils.run_bass_kernel_spmd(nc, in_maps, core_ids=list(range(NC)))
    out = np.concatenate([np.asarray(res.results[c]['outT']).T
                          for c in range(NC)], axis=0)
    return np.ascontiguousarray(out, np.float32)


def timed_run(tmpdir=None, **inputs):
    """Run with NTFF profiling; returns BassKernelResults."""
    from concourse import bass_utils
    nc = _compiled(T)
    in_maps = _make_in_maps(inputs)
    res = bass_utils.run_bass_kernel_spmd(
        nc, in_maps, core_ids=list(range(NC)), trace=True, tmpdir=tmpdir)
    return res



# revision 28
# speedup vs baseline: 1.8356x; 1.1744x over previous
"""Trainium2 Bass kernel for stacked-LSTM + attention + dense head (v2).

Model (per reference):
  3x LSTM(H=512, return_sequences) with inference BatchNorm between layers,
  attention pooling over time, then Dense(128)+BN+Dense(64)+Dense(5).
  B=128, T=512, D=128, H=512, fp32.

Strategy: data-parallel over batch (16 rows/core on 8 cores). Per core a
3-layer wavefront with lags 16/32 (ring-aligned). Key structure vs v1:
  - xz (input projection) is INJECTED into PSUM by K=16 identity matmuls
    (plus the recurrent matmuls accumulate on top), removing the wide DVE
    adds from the critical chain.
  - Biases enter via K=1 ones-row matmuls (no DVE bias adds anywhere).
  - Gates use an interleaved column permutation [i_n|f_n|o_n|g_n] per
    512-chunk so sigmoid/tanh run per-chunk, hidden under the MM stream.
  - Projections for layers 1/2 run on PE column strip 3 (2 steps/tile),
    evicted by a single tensor_copy and DMA'd into a 16-slot SBUF ring.
  - The c/h elementwise chain is split across DVE and GpSimd.
  - h transpose via DVE StreamTranspose + 12 small scatter DMAs spread
    over 4 issuing engines.

Self-contained: hardcodes shapes; no reads of reference.py/spec.json.
"""

import functools

import numpy as np

B, T, D, H = 128, 512, 128, 512
NC = 8
BL = B // NC          # batch rows per core
G4 = 4 * H            # gate width 2048
EPS = 1e-3
P = 128
LAG1, LAG2 = 16, 32   # wavefront lags (multiples of RING)
RING = 16             # xz ring slots
HR = 16               # hT time-ring length

# column permutation: keras gate order [i|f|g|o] (512 each) ->
# kernel order: per 512-chunk n: [i_n | f_n | o_n | g_n] (128 each)
_PERM = np.concatenate([
    np.concatenate([
        np.arange(0 + 128 * n, 128 + 128 * n),        # i_n
        np.arange(512 + 128 * n, 640 + 128 * n),      # f_n
        np.arange(1536 + 128 * n, 1664 + 128 * n),    # o_n
        np.arange(1024 + 128 * n, 1152 + 128 * n),    # g_n
    ]) for n in range(4)
])
# gate col indices in permuted layout
_I_COLS = np.concatenate([np.arange(n * 512, n * 512 + 128) for n in range(4)])
_F_COLS = _I_COLS + 128
_O_COLS = _I_COLS + 256
_G_COLS = _I_COLS + 384


def _bn_fold(g, b, m, v):
    sc = g / np.sqrt(v + EPS)
    sh = b - m * sc
    return sc.astype(np.float32), sh.astype(np.float32)


def _bf16(a):
    import ml_dtypes
    return np.ascontiguousarray(np.asarray(a, np.float32).astype(
        ml_dtypes.bfloat16))


def _f8(a):
    import ml_dtypes
    return np.ascontiguousarray(np.asarray(a, np.float32).astype(
        ml_dtypes.float8_e4m3fn))


def prep_weights(inp):
    """Host-side constant folding. Returns dict of prepared arrays."""
    f = np.float32
    o = {}
    o['W0p'] = np.ascontiguousarray(inp['W0'][:, _PERM], f)
    o['b0row'] = np.ascontiguousarray(inp['b0'][_PERM], f)
    o['U0b'] = _f8(inp['U0'][:, _PERM])
    o['U1b'] = _f8(inp['U1'][:, _PERM])
    o['U2b'] = _f8(inp['U2'][:, _PERM])
    sc0, sh0 = _bn_fold(inp['bn0_g'], inp['bn0_b'], inp['bn0_m'], inp['bn0_v'])
    o['W1b'] = _f8((sc0[:, None] * inp['W1'])[:, _PERM])
    o['b1b'] = _bf16((inp['b1'] + sh0 @ inp['W1'])[_PERM])
    sc1, sh1 = _bn_fold(inp['bn1_g'], inp['bn1_b'], inp['bn1_m'], inp['bn1_v'])
    o['W2b'] = _f8((sc1[:, None] * inp['W2'])[:, _PERM])
    o['b2b'] = _bf16((inp['b2'] + sh1 @ inp['W2'])[_PERM])
    o['Wab'] = _f8(inp['Wa'])
    o['ba'] = np.ascontiguousarray(inp['ba'], f)
    # pooled = sum_t a*h2 (no 1/T); fold 1/T into Wd1
    o['Wd1p'] = np.ascontiguousarray(inp['Wd1'] / np.float32(T), f)
    o['bd1'] = np.ascontiguousarray(inp['bd1'], f)
    sc2, sh2 = _bn_fold(inp['bn2_g'], inp['bn2_b'], inp['bn2_m'], inp['bn2_v'])
    o['Wd2p'] = np.ascontiguousarray(sc2[:, None] * inp['Wd2'], f)
    o['bd2p'] = np.ascontiguousarray(inp['bd2'] + sh2 @ inp['Wd2'], f)
    o['Wd3'] = np.ascontiguousarray(inp['Wd3'], f)
    o['bd3'] = np.ascontiguousarray(inp['bd3'], f)
    # selector for summing rows (t,b) -> b : sel[p, b] = 1 if p % BL == b
    sel = np.zeros((P, BL), f)
    sel[np.arange(P), np.arange(P) % BL] = 1.0
    o['sel'] = sel
    o['ident'] = np.eye(P, dtype=f)
    eye80sq = np.zeros((80, 80), f)
    for l in range(3):
        sl = slice(32 * l, 32 * l + BL)
        eye80sq[sl, sl] = np.eye(BL, dtype=f)
    o['eye80sq'] = _bf16(eye80sq)
    o['ones1'] = np.ones((1, P), f)
    o['ones32b'] = _bf16(np.ones((1, 32), f))
    return o


def _sigmoid(x):
    return 1.0 / (1.0 + np.exp(-x))


def numpy_forward(inp, t_steps=T, b_rows=B):
    """Numpy mirror of the kernel math (folded weights, permuted gates),
    in fp32 (no bf16 effects). Validates the host-side folds."""
    w = prep_weights(inp)
    x = np.asarray(inp['x'], np.float32)[:b_rows, :t_steps]
    U = {0: np.asarray(w['U0b'], np.float32),
         1: np.asarray(w['U1b'], np.float32),
         2: np.asarray(w['U2b'], np.float32)}
    W1 = np.asarray(w['W1b'], np.float32)
    W2 = np.asarray(w['W2b'], np.float32)

    def scan(xz, Um):
        bsz = xz.shape[0]
        h = np.zeros((bsz, H), np.float32)
        c = np.zeros((bsz, H), np.float32)
        hs = np.empty((bsz, t_steps, H), np.float32)
        for t in range(t_steps):
            z = xz[:, t] + h @ Um
            i = _sigmoid(z[:, _I_COLS]); f = _sigmoid(z[:, _F_COLS])
            o_ = _sigmoid(z[:, _O_COLS]); g = np.tanh(z[:, _G_COLS])
            c = f * c + i * g
            h = o_ * np.tanh(c)
            hs[:, t] = h
        return hs  # [B, T, H]

    xz0 = np.einsum('btd,dg->btg', x, w['W0p']) + w['b0row']
    h0 = scan(xz0, U[0])
    xz1 = np.einsum('bth,hg->btg', h0, W1) + np.asarray(w['b1b'], np.float32)
    h1 = scan(xz1, U[1])
    xz2 = np.einsum('bth,hg->btg', h1, W2) + np.asarray(w['b2b'], np.float32)
    h2 = scan(xz2, U[2])

    e = np.tanh(np.einsum('bth,hk->btk', h2, np.asarray(w['Wab'], np.float32))
                + w['ba'])
    s = e.sum(-1)
    s = s - s.max(axis=1, keepdims=True)
    a = np.exp(s); a = a / a.sum(axis=1, keepdims=True)
    pooled = np.einsum('bt,bth->bh', a, h2)
    d1 = np.maximum(pooled @ w['Wd1p'] + w['bd1'], 0)
    d2 = np.maximum(d1 @ w['Wd2p'] + w['bd2p'], 0)
    return d2 @ w['Wd3'] + w['bd3']


# ---------------------------------------------------------------------------
# Bass program
# ---------------------------------------------------------------------------

def build_nc(t_steps=T):
    import concourse.bacc as bacc
    import concourse.mybir as mybir
    import concourse.tile as tile
    from contextlib import ExitStack

    f32 = mybir.dt.float32
    f32r = mybir.dt.float32r
    bf16 = mybir.dt.bfloat16
    f8 = mybir.dt.float8e4
    AF = mybir.ActivationFunctionType
    OP = mybir.AluOpType
    PM = mybir.MatmulPerfMode
    M = t_steps * BL
    MT = M // P
    TPB = P // BL  # timesteps per 128-row tile (8)
    NSLOT = t_steps + LAG2

    nc = bacc.Bacc("TRN2", target_bir_lowering=False, debug=False,
                   num_devices=NC)

    def din(name, shape, dt=f32):
        return nc.dram_tensor(name, list(shape), dt, kind="ExternalInput")

    x_d = din('xT', (D, t_steps, BL))
    W0p = din('W0p', (D, G4)); b0row = din('b0row', (G4,))
    U_d = [din('U0b', (H, G4), f8), din('U1b', (H, G4), f8),
           din('U2b', (H, G4), f8)]
    W_d = {1: din('W1b', (H, G4), f8), 2: din('W2b', (H, G4), f8)}
    brow_d = {1: din('b1b', (G4,), bf16), 2: din('b2b', (G4,), bf16)}
    Wab = din('Wab', (H, H), f8); ba = din('ba', (H,))
    Wd1p = din('Wd1p', (H, P)); bd1 = din('bd1', (P,))
    Wd2p = din('Wd2p', (P, 64)); bd2p = din('bd2p', (64,))
    Wd3 = din('Wd3', (64, 5)); bd3 = din('bd3', (5,))
    sel_d = din('sel', (P, BL))
    ident_d = din('ident', (P, P))
    eye80sq_d = din('eye80sq', (80, 80), bf16)
    ones1_d = din('ones1', (1, P))
    ones32_d = din('ones32b', (1, 32), bf16)
    outT = nc.dram_tensor('outT', [5, BL], f32, kind="ExternalOutput")

    # DRAM temps
    xz_d = nc.dram_tensor('xz_buf', [M, G4], bf16)
    h2T = nc.dram_tensor('h2T', [4, P, t_steps, BL], f8)
    h2rows = nc.dram_tensor('h2rows', [M, H], f8)
    s_dram = nc.dram_tensor('s_dram', [M], f32)
    a_dram = nc.dram_tensor('a_dram', [M], f32)

    NSL = [slice(n * 512, (n + 1) * 512) for n in range(4)]
    ROWS = [slice(32 * l, 32 * l + BL) for l in range(3)]

    with tile.TileContext(nc) as tc:
        with ExitStack() as gctx:
            gconst = gctx.enter_context(tc.tile_pool(name="gconst", bufs=1))
            ident = gconst.tile([P, P], f32)
            nc.sync.dma_start(ident[:], ident_d[:, :])
            sel = gconst.tile([P, BL], f32)
            nc.sync.dma_start(sel[:], sel_d[:, :])

            # ---------------- layer-0 input projection pass ----------------
            def xz_pass():
                with ExitStack() as ctx:
                    cst = ctx.enter_context(tc.tile_pool(name="p0c", bufs=1))
                    W_stg = cst.tile([P, G4], f32, name="p0Ws")
                    nc.sync.dma_start(W_stg[:], W0p[:, :])
                    W_sb = cst.tile([P, G4], f32r, name="p0W")
                    nc.any.tensor_copy(W_sb[:], W_stg[:])
                    ones_s = cst.tile([1, P], f32, name="p0o_s")
                    nc.sync.dma_start(ones_s[:], ones1_d[:, :])
                    ones_sb = cst.tile([1, P], f32r, name="p0o")
                    nc.any.tensor_copy(ones_sb[:], ones_s[:])
                    b0_s = cst.tile([1, G4], f32, name="p0b_s")
                    nc.sync.dma_start(b0_s[:], b0row[None, :])
                    b0_sb = cst.tile([1, G4], f32r, name="p0b")
                    nc.any.tensor_copy(b0_sb[:], b0_s[:])
                    io = ctx.enter_context(tc.tile_pool(name="p0io", bufs=3))
                    ps = ctx.enter_context(
                        tc.tile_pool(name="p0ps", bufs=2, space="PSUM"))
                    for m in range(MT):
                        km_s = io.tile([P, P], f32, tag="km_s")
                        nc.sync.dma_start(
                            km_s[:].rearrange("p (t b) -> p t b", b=BL),
                            x_d[:, m * TPB:(m + 1) * TPB, :])
                        km = io.tile([P, P], f32r, tag="km")
                        nc.any.tensor_copy(km[:], km_s[:])
                        zp = ps.tile([P, G4], f32, tag="zp")
                        for n in range(4):
                            nc.tensor.matmul(zp[:, NSL[n]], ones_sb[:],
                                             b0_sb[:, NSL[n]],
                                             start=True, stop=False)
                            nc.tensor.matmul(zp[:, NSL[n]], km[:],
                                             W_sb[:, NSL[n]],
                                             start=False, stop=True)
                        ob = io.tile([P, G4], bf16, tag="ob")
                        nc.vector.tensor_copy(ob[:], zp[:])
                        nc.sync.dma_start(xz_d[m * P:(m + 1) * P, :], ob[:])

            # ---------------- 3-layer wavefront scan ----------------
            def wavefront():
                with ExitStack() as ctx:
                    cst = ctx.enter_context(tc.tile_pool(name="wfc", bufs=1))
                    # fp8 weights in DoubleRow layout: [p, c, i, n] where
                    # contraction row = 256c + 128i + p
                    U_sb = []
                    for l in range(3):
                        u = cst.tile([P, 2, 2, G4], f8, name=f"wfU{l}")
                        nc.sync.dma_start(
                            u[:], U_d[l].rearrange("(c i p) n -> p c i n",
                                                   c=2, i=2))
                        U_sb.append(u)
                    W_sb = {}
                    for l in (1, 2):
                        w = cst.tile([P, 2, 2, G4], f8, name=f"wfW{l}")
                        nc.sync.dma_start(
                            w[:], W_d[l].rearrange("(c i p) n -> p c i n",
                                                   c=2, i=2))
                        W_sb[l] = w
                    brow = {}
                    for l in (1, 2):
                        bt = cst.tile([1, G4], bf16, name=f"wfbr{l}")
                        nc.sync.dma_start(bt[:], brow_d[l][None, :])
                        brow[l] = bt
                    eye80 = cst.tile([80, 80], bf16, name="wfeye")
                    nc.sync.dma_start(eye80[:], eye80sq_d[:, :])
                    ones32 = cst.tile([1, 32], bf16, name="wfo32")
                    nc.sync.dma_start(ones32[:], ones32_d[:, :])
                    # persistent state: c split per chunk-half (tile-granular
                    # dep tracking — separate tiles avoid false WAR/RAW)
                    c_h = []
                    for hh in range(2):
                        t_ = cst.tile([80, 2, P], f32, name=f"wf_c{hh}")
                        nc.vector.memset(t_[:], 0.0)
                        c_h.append(t_)
                    # double-buffered (slot parity) per-half gate temps
                    cf_h = [[cst.tile([80, 2, P], f32, name=f"wf_cf{p}{hh}")
                             for hh in range(2)] for p in range(2)]
                    ig_h = [[cst.tile([80, 2, P], bf16, name=f"wf_ig{p}{hh}")
                             for hh in range(2)] for p in range(2)]
                    tch_h = [[cst.tile([80, 2, P], bf16, name=f"wf_tc{p}{hh}")
                              for hh in range(2)] for p in range(2)]
                    sig_h = [[cst.tile([80, 2, 384], bf16,
                                       name=f"wf_sg{p}{hh}")
                              for hh in range(2)] for p in range(2)]
                    gb_h = [[cst.tile([80, 2, P], bf16, name=f"wf_g{p}{hh}")
                             for hh in range(2)] for p in range(2)]
                    h_pp = []
                    tmp_pp = []
                    for p in range(2):
                        hb = cst.tile([96, H], f8, name=f"wf_h{p}")
                        nc.vector.memset(hb[:], 0.0)
                        h_pp.append(hb)
                        tb = cst.tile([96, H], f8, name=f"wf_tmp{p}")
                        nc.vector.memset(tb[:], 0.0)
                        tmp_pp.append(tb)
                    hT = [cst.tile([P, 4, HR, BL], f8, name=f"wfhT{l}")
                          for l in range(3)]
                    for l in range(3):
                        nc.vector.memset(hT[l][:], 0.0)
                    ring = cst.tile([80, RING, G4], bf16, name="wf_ring")
                    nc.vector.memset(ring[:], 0.0)
                    psp = ctx.enter_context(
                        tc.tile_pool(name="wfps", bufs=1, space="PSUM"))
                    # one PSUM tile per (parity, chunk-pair): 4 x 2 banks
                    Zpp = []
                    for p in range(2):
                        row = []
                        for hh in range(2):
                            t_ = psp.tile([P, 1024], f32,
                                          name=f"wf_z{p}{hh}")
                            nc.vector.memset(t_[:], 0.0)
                            row.append(t_)
                        Zpp.append(row)
                    wk = ctx.enter_context(tc.tile_pool(name="wfwk", bufs=3))

                    def prefetch_xz0(t0):
                        # load xz rows for steps [t0, t0+4) into ring rows 0:16
                        if 0 <= t0 < t_steps:
                            nt = min(4, t_steps - t0)
                            nc.gpsimd.dma_start(
                                ring[0:BL, t0 % RING:t0 % RING + nt, :],
                                xz_d[t0 * BL:(t0 + nt) * BL, :]
                                .rearrange("(t b) g -> b t g", b=BL))

                    prefetch_xz0(0)

                    SCAT_ENG = [nc.sync, nc.gpsimd, nc.scalar]
                    pj_state = {}  # live pj tile: (L, tau, stage)

                    def emit_pj_half(L, tau, hh, z):
                        """Emit pj matmuls for chunk pair hh into z rows
                        96:128 (DoubleRow fp8; evicted at slot end)."""
                        r0 = tau % HR
                        for q in range(2):
                            n = 2 * hh + q
                            nc.tensor.matmul(
                                z[hh][96:128, 512 * q:512 * (q + 1)],
                                ones32[:], brow[L][:, NSL[n]],
                                start=True, stop=False,
                                tile_position=(0, 96))
                        for c2 in range(2):
                            stat = hT[L - 1][:, 2 * c2:2 * c2 + 2,
                                             r0:r0 + 2, :]
                            for q in range(2):
                                n = 2 * hh + q
                                nc.tensor.matmul(
                                    z[hh][96:128, 512 * q:512 * (q + 1)],
                                    stat, W_sb[L][:, c2, :, NSL[n]],
                                    start=False, stop=(c2 == 1),
                                    perf_mode=PM.DoubleRow,
                                    tile_position=(0, 96))

                    for s in range(NSLOT):
                        ts_ = [s, s - LAG1, s - LAG2]
                        act = [0 <= t < t_steps for t in ts_]
                        rec = [act[l] and ts_[l] >= 1 for l in range(3)]
                        any_rec = any(rec)
                        r = s % RING
                        par = s % 2
                        z = Zpp[par]          # this slot's gate banks
                        sigb = sig_h[par]; gb = gb_h[par]; tch = tch_h[par]
                        cfp = cf_h[par]; igp = ig_h[par]
                        h_bf = h_pp[par]; tmp_bf = tmp_pp[par]

                        # ---- pj matmuls first: they stream on PE strip 3
                        # concurrently with this slot's gate matmuls and
                        # read only old hT ring positions.
                        fin = 2 if s % 2 == 0 else 1
                        pj_fin = pj_state.pop(fin, None)
                        if pj_fin is not None:
                            L, tau, stage = pj_fin
                            emit_pj_half(L, tau, 1, z)
                        beg = 1 if s % 2 == 0 else 2
                        tau_b = s - 4 if beg == 1 else s - 17
                        pj_beg = None
                        if 0 <= tau_b <= t_steps - 2:
                            stage = wk.tile([32, G4], bf16, tag="stage")
                            pj_state[beg] = (beg, tau_b, stage)
                            pj_beg = pj_state[beg]
                            emit_pj_half(beg, tau_b, 0, z)

                        # ---- per-half tail: tanh(c), h, transpose ----
                        def tail(hh):
                            nc.scalar.activation(
                                tch[hh][:], c_h[hh][:], AF.Tanh)
                            nc.vector.tensor_tensor(
                                h_bf[0:80, 256 * hh:256 * (hh + 1)]
                                .rearrange("p (q c) -> p q c", q=2),
                                sigb[hh][:, :, 256:384], tch[hh][:],
                                OP.mult)
                            nc.vector.transpose(
                                tmp_bf[:, 256 * hh:256 * (hh + 1)],
                                h_bf[:, 256 * hh:256 * (hh + 1)])

                        # ---- per-half: inject + recurrent MMs + gates ----
                        for hh in range(2):
                            for q in range(2):
                                n = 2 * hh + q
                                nc.tensor.matmul(
                                    z[hh][0:80, 512 * q:512 * (q + 1)],
                                    eye80[:], ring[0:80, r, NSL[n]],
                                    start=True, stop=not any_rec,
                                    skip_group_check=True)
                                for c2 in range(2):
                                    for l in range(3):
                                        if rec[l]:
                                            nc.tensor.matmul(
                                                z[hh][32 * l:32 * l + BL,
                                                      512 * q:512 * (q + 1)],
                                                hT[l][:, 2 * c2:2 * c2 + 2,
                                                      (ts_[l] - 1) % HR, :],
                                                U_sb[l][:, c2, :, NSL[n]],
                                                start=False, stop=(c2 == 1),
                                                perf_mode=PM.DoubleRow,
                                                tile_position=(0, 32 * l),
                                                skip_group_check=True)
                            # gates for chunk pair hh
                            nc.scalar.activation(
                                sigb[hh][:],
                                z[hh][0:80, :]
                                .rearrange("p (q c) -> p q c", q=2)
                                [:, :, 0:384],
                                AF.Sigmoid)
                            nc.scalar.activation(
                                gb[hh][:],
                                z[hh][0:80, :]
                                .rearrange("p (q c) -> p q c", q=2)
                                [:, :, 384:512],
                                AF.Tanh)
                            nc.vector.tensor_tensor(
                                cfp[hh][:], c_h[hh][:],
                                sigb[hh][:, :, 128:256], OP.mult)
                            nc.gpsimd.tensor_tensor(
                                igp[hh][:], sigb[hh][:, :, 0:128],
                                gb[hh][:], OP.mult)
                            nc.vector.tensor_tensor(
                                c_h[hh][:], cfp[hh][:], igp[hh][:], OP.add)
                            if hh == 1:
                                tail(0)
                        tail(1)

                        # ---- scatter transposed h into hT rings ----
                        for l in range(3):
                            if not act[l]:
                                continue
                            for j in range(4):
                                SCAT_ENG[(j + l) % 3].dma_start(
                                    hT[l][32 * j:32 * j + 32, :,
                                          ts_[l] % HR, :],
                                    tmp_bf[32 * l:32 * l + 32, :]
                                    .rearrange("p (k c) -> p k c", c=P)
                                    [:, :, 32 * j:32 * j + BL])

                        # ---- layer-2 outputs for attention ----
                        if act[2]:
                            t2 = ts_[2]
                            nc.sync.dma_start(
                                h2rows[t2 * BL:(t2 + 1) * BL, :],
                                h_bf[64:64 + BL, :])
                            if t2 % 4 == 3:
                                r4 = (t2 - 3) % HR
                                nc.sync.dma_start(
                                    h2T.rearrange("k p t b -> p k (t b)")
                                    [:, :, (t2 - 3) * BL:(t2 + 1) * BL],
                                    hT[2][:, :, r4:r4 + 4, :]
                                    .rearrange("p k t b -> p k (t b)"))

                        # ---- projection work on strip 3, 2-phase ----
                        # Emitted at slot END so pj MMs fill the PE while
                        # this slot's gate chain drains, and pj evictions
                        # (DVE) queue after the gate ops.
                        # ---- pj evictions (lazy: inject reuses these rows
                        # only two slots later) + ring stores ----
                        if pj_fin is not None:
                            L, tau, stage = pj_fin
                            nc.vector.tensor_copy(
                                stage[:, G4 // 2:], z[1][96:128, :])
                            r0 = tau % HR
                            for i2 in range(2):
                                nc.scalar.dma_start(
                                    ring[32 * L:32 * L + BL, r0 + i2, :],
                                    stage[BL * i2:BL * (i2 + 1), :])
                        if pj_beg is not None:
                            nc.scalar.activation(
                                pj_beg[2][:, 0:G4 // 2], z[0][96:128, :],
                                AF.Identity)
                        if s % 4 == 0:
                            prefetch_xz0(s + 4)

            # ---------------- run pipeline ----------------
            xz_pass()
            wavefront()

            # ---------------- attention ----------------
            with ExitStack() as ctx:
                cst = ctx.enter_context(tc.tile_pool(name="atc", bufs=1))
                Wa_sb = cst.tile([P, 4, H], f8, name="atWa")
                nc.sync.dma_start(
                    Wa_sb[:], Wab.rearrange("(k p) n -> p k n", p=P))
                ba_rep = cst.tile([P, H], f32)
                nc.sync.dma_start(ba_rep[:], ba[None, :].to_broadcast((P, H)))
                s_sb = cst.tile([P, MT], f32)
                io = ctx.enter_context(tc.tile_pool(name="atio", bufs=3))
                ps = ctx.enter_context(
                    tc.tile_pool(name="atps", bufs=2, space="PSUM"))
                # e-pass: s[(t,b)] = sum_k tanh(h2 @ Wa + ba)
                for m in range(MT):
                    kxm = io.tile([P, 4, TPB, BL], bf16, tag="kxm")
                    for k in range(4):
                        nc.sync.dma_start(
                            kxm[:, k],
                            h2T[k, :, m * TPB:(m + 1) * TPB, :])
                    ep = ps.tile([P, H], f32, tag="ep")
                    for k in range(4):
                        nc.tensor.matmul(
                            ep[:], kxm[:, k], Wa_sb[:, k, :],
                            start=(k == 0), stop=(k == 3))
                    e_sb = io.tile([P, H], f32, tag="e")
                    nc.vector.tensor_tensor(e_sb[:], ep[:], ba_rep[:], OP.add)
                    e_t = io.tile([P, H], f32, tag="et")
                    nc.scalar.activation(e_t[:], e_sb[:], AF.Tanh,
                                         accum_out=s_sb[:, m:m + 1])

                # s (row layout [P, MT]) -> sT [BL, t_steps] via flat DRAM
                nc.sync.dma_start(
                    s_dram.rearrange("(m p) -> p m", p=P), s_sb[:])
                sT = cst.tile([BL, t_steps], f32)
                nc.sync.dma_start(
                    sT[:], s_dram.rearrange("(t b) -> b t", b=BL))
                mx = cst.tile([BL, 1], f32)
                nc.vector.reduce_max(mx[:], sT[:], axis=mybir.AxisListType.X)
                nmx = cst.tile([BL, 1], f32)
                nc.vector.tensor_scalar_mul(nmx[:], mx[:], -1.0)
                ex = cst.tile([BL, t_steps], f32)
                sm = cst.tile([BL, 1], f32)
                nc.scalar.activation(ex[:], sT[:], AF.Exp, bias=nmx[:],
                                     accum_out=sm[:])
                rs = cst.tile([BL, 1], f32)
                nc.vector.reciprocal(rs[:], sm[:])
                aT = cst.tile([BL, t_steps], f32)
                nc.vector.tensor_scalar_mul(aT[:], ex[:], rs[:])
                nc.sync.dma_start(
                    a_dram.rearrange("(t b) -> b t", b=BL), aT[:])
                a_row = cst.tile([P, MT], f32)
                nc.sync.dma_start(
                    a_row[:], a_dram.rearrange("(m p) -> p m", p=P))

                # pooled[b, :] = sum_rows sel * (a * h2)   (f32 matmuls)
                pp = ctx.enter_context(
                    tc.tile_pool(name="atpp", bufs=1, space="PSUM"))
                ps1 = ctx.enter_context(
                    tc.tile_pool(name="atp1", bufs=1, space="PSUM"))
                pooled_ps = pp.tile([BL, H], f32)
                for m in range(MT):
                    h2t = io.tile([P, H], bf16, tag="h2t")
                    nc.sync.dma_start(h2t[:], h2rows[m * P:(m + 1) * P, :])
                    wrow = io.tile([P, H], f32, tag="wrow")
                    nc.vector.tensor_scalar_mul(wrow[:], h2t[:],
                                                a_row[:, m:m + 1])
                    nc.tensor.matmul(pooled_ps[:], sel[:], wrow[:],
                                     start=(m == 0), stop=(m == MT - 1))

                # pooledT via PE transpose
                pooled_sb = cst.tile([BL, H], f32)
                nc.vector.tensor_copy(pooled_sb[:], pooled_ps[:])
                ptp = ps1.tile([P, 4 * BL], f32, tag="ptp")
                for k in range(4):
                    nc.tensor.transpose(
                        ptp[:, k * BL:(k + 1) * BL],
                        pooled_sb[:, k * P:(k + 1) * P], ident[0:BL, 0:BL])
                pooledT = cst.tile([P, 4, BL], f32r)
                nc.vector.tensor_copy(
                    pooledT[:], ptp[:].rearrange("p (k b) -> p k b", k=4))

                # ---------------- dense head ----------------
                def load_r(pool, dram_ap, shape, name):
                    stg = pool.tile(shape, f32, name=name + "_stg")
                    nc.sync.dma_start(stg[:], dram_ap)
                    t_ = pool.tile(shape, f32r, name=name)
                    nc.any.tensor_copy(t_[:], stg[:])
                    return t_

                Wd1_sb = load_r(cst, Wd1p.rearrange("(k p) n -> p k n", p=P),
                                [P, 4, P], "hWd1")
                bd1_sb = cst.tile([P, 1], f32)
                nc.sync.dma_start(bd1_sb[:], bd1[:, None])
                Wd2_sb = load_r(cst, Wd2p[:, :], [P, 64], "hWd2")
                bd2_sb = cst.tile([64, 1], f32)
                nc.sync.dma_start(bd2_sb[:], bd2p[:, None])
                Wd3_sb = load_r(cst, Wd3[:, :], [64, 5], "hWd3")
                bd3_sb = cst.tile([5, 1], f32)
                nc.sync.dma_start(bd3_sb[:], bd3[:, None])

                d1p = ps1.tile([P, BL], f32, tag="d1p")
                for k in range(4):
                    nc.tensor.matmul(d1p[:], Wd1_sb[:, k, :], pooledT[:, k, :],
                                     start=(k == 0), stop=(k == 3))
                d1 = cst.tile([P, BL], f32r)
                nc.scalar.activation(d1[:], d1p[:], AF.Relu, bias=bd1_sb[:])
                d2p = ps1.tile([64, BL], f32, tag="d2p")
                nc.tensor.matmul(d2p[:], Wd2_sb[:], d1[:], start=True,
                                 stop=True)
                d2 = cst.tile([64, BL], f32r)
                nc.scalar.activation(d2[:], d2p[:], AF.Relu, bias=bd2_sb[:])
                d3p = ps1.tile([5, BL], f32, tag="d3p")
                nc.tensor.matmul(d3p[:], Wd3_sb[:], d2[:], start=True,
                                 stop=True)
                d3 = cst.tile([5, BL], f32)
                nc.scalar.activation(d3[:], d3p[:], AF.Identity, bias=bd3_sb[:])
                nc.sync.dma_start(outT[:, :], d3[:])

    nc.compile()
    return nc


@functools.lru_cache(maxsize=2)
def _compiled(t_steps):
    return build_nc(t_steps)


def _make_in_maps(inputs):
    w = prep_weights(inputs)
    x = np.ascontiguousarray(np.asarray(inputs['x'], np.float32))
    base = {k: w[k] for k in (
        'W0p', 'b0row', 'U0b', 'U1b', 'U2b', 'W1b', 'b1b', 'W2b', 'b2b',
        'Wab', 'ba', 'Wd1p', 'bd1', 'Wd2p', 'bd2p', 'Wd3', 'bd3', 'sel',
        'ident', 'eye80sq', 'ones1', 'ones32b')}
    in_maps = []
    for c in range(NC):
        m = dict(base)
        m['xT'] = np.ascontiguousarray(
            x[c * BL:(c + 1) * BL].transpose(2, 1, 0))
        in_maps.append(m)
    return in_maps


def kernel(**inputs):
    from concourse import bass_utils
    nc = _compiled(T)
    in_maps = _make_in_maps(inputs)
    res = bass_utils.run_bass_kernel_spmd(nc, in_maps, core_ids=list(range(NC)))
    out = np.concatenate([np.asarray(res.results[c]['outT']).T
                          for c in range(NC)], axis=0)
    return np.ascontiguousarray(out, np.float32)


def timed_run(tmpdir=None, **inputs):
    """Run with NTFF profiling; returns BassKernelResults."""
    from concourse import bass_utils
    nc = _compiled(T)
    in_maps = _make_in_maps(inputs)
    res = bass_utils.run_bass_kernel_spmd(
        nc, in_maps, core_ids=list(range(NC)), trace=True, tmpdir=tmpdir)
    return res



# revision 36
# speedup vs baseline: 1.9331x; 1.0531x over previous
"""Trainium2 Bass kernel for stacked-LSTM + attention + dense head (v2).

Model (per reference):
  3x LSTM(H=512, return_sequences) with inference BatchNorm between layers,
  attention pooling over time, then Dense(128)+BN+Dense(64)+Dense(5).
  B=128, T=512, D=128, H=512, fp32.

Strategy: data-parallel over batch (16 rows/core on 8 cores). Per core a
3-layer wavefront with lags 16/32 (ring-aligned). Key structure vs v1:
  - xz (input projection) is INJECTED into PSUM by K=16 identity matmuls
    (plus the recurrent matmuls accumulate on top), removing the wide DVE
    adds from the critical chain.
  - Biases enter via K=1 ones-row matmuls (no DVE bias adds anywhere).
  - Gates use an interleaved column permutation [i_n|f_n|o_n|g_n] per
    512-chunk so sigmoid/tanh run per-chunk, hidden under the MM stream.
  - Projections for layers 1/2 run on PE column strip 3 (2 steps/tile),
    evicted by a single tensor_copy and DMA'd into a 16-slot SBUF ring.
  - The c/h elementwise chain is split across DVE and GpSimd.
  - h transpose via DVE StreamTranspose + 12 small scatter DMAs spread
    over 4 issuing engines.

Self-contained: hardcodes shapes; no reads of reference.py/spec.json.
"""

import functools

import numpy as np

B, T, D, H = 128, 512, 128, 512
NC = 8
BL = B // NC          # batch rows per core
G4 = 4 * H            # gate width 2048
EPS = 1e-3
P = 128
LAG1, LAG2 = 16, 32   # wavefront lags (multiples of RING)
RING = 16             # xz ring slots
HR = 16               # hT time-ring length

# column permutation: keras gate order [i|f|g|o] (512 each) ->
# kernel order: per 512-chunk n: [i_n | f_n | o_n | g_n] (128 each)
_PERM = np.concatenate([
    np.concatenate([
        np.arange(0 + 128 * n, 128 + 128 * n),        # i_n
        np.arange(512 + 128 * n, 640 + 128 * n),      # f_n
        np.arange(1536 + 128 * n, 1664 + 128 * n),    # o_n
        np.arange(1024 + 128 * n, 1152 + 128 * n),    # g_n
    ]) for n in range(4)
])
# gate col indices in permuted layout
_I_COLS = np.concatenate([np.arange(n * 512, n * 512 + 128) for n in range(4)])
_F_COLS = _I_COLS + 128
_O_COLS = _I_COLS + 256
_G_COLS = _I_COLS + 384


def _bn_fold(g, b, m, v):
    sc = g / np.sqrt(v + EPS)
    sh = b - m * sc
    return sc.astype(np.float32), sh.astype(np.float32)


def _bf16(a):
    import ml_dtypes
    return np.ascontiguousarray(np.asarray(a, np.float32).astype(
        ml_dtypes.bfloat16))


def _f8(a):
    import ml_dtypes
    return np.ascontiguousarray(np.asarray(a, np.float32).astype(
        ml_dtypes.float8_e4m3fn))


def prep_weights(inp):
    """Host-side constant folding. Returns dict of prepared arrays."""
    f = np.float32
    o = {}
    o['W0p'] = np.ascontiguousarray(inp['W0'][:, _PERM], f)
    o['b0row'] = np.ascontiguousarray(inp['b0'][_PERM], f)
    o['U0b'] = _f8(inp['U0'][:, _PERM])
    o['U1b'] = _f8(inp['U1'][:, _PERM])
    o['U2b'] = _f8(inp['U2'][:, _PERM])
    sc0, sh0 = _bn_fold(inp['bn0_g'], inp['bn0_b'], inp['bn0_m'], inp['bn0_v'])
    o['W1b'] = _f8((sc0[:, None] * inp['W1'])[:, _PERM])
    o['b1b'] = _bf16((inp['b1'] + sh0 @ inp['W1'])[_PERM])
    sc1, sh1 = _bn_fold(inp['bn1_g'], inp['bn1_b'], inp['bn1_m'], inp['bn1_v'])
    o['W2b'] = _f8((sc1[:, None] * inp['W2'])[:, _PERM])
    o['b2b'] = _bf16((inp['b2'] + sh1 @ inp['W2'])[_PERM])
    o['Wab'] = _f8(inp['Wa'])
    o['ba'] = np.ascontiguousarray(inp['ba'], f)
    # pooled = sum_t a*h2 (no 1/T); fold 1/T into Wd1
    o['Wd1p'] = np.ascontiguousarray(inp['Wd1'] / np.float32(T), f)
    o['bd1'] = np.ascontiguousarray(inp['bd1'], f)
    sc2, sh2 = _bn_fold(inp['bn2_g'], inp['bn2_b'], inp['bn2_m'], inp['bn2_v'])
    o['Wd2p'] = np.ascontiguousarray(sc2[:, None] * inp['Wd2'], f)
    o['bd2p'] = np.ascontiguousarray(inp['bd2'] + sh2 @ inp['Wd2'], f)
    o['Wd3'] = np.ascontiguousarray(inp['Wd3'], f)
    o['bd3'] = np.ascontiguousarray(inp['bd3'], f)
    # selector for summing rows (t,b) -> b : sel[p, b] = 1 if p % BL == b
    sel = np.zeros((P, BL), f)
    sel[np.arange(P), np.arange(P) % BL] = 1.0
    o['sel'] = sel
    o['ident'] = np.eye(P, dtype=f)
    eye80sq = np.zeros((80, 80), f)
    for l in range(3):
        sl = slice(32 * l, 32 * l + BL)
        eye80sq[sl, sl] = np.eye(BL, dtype=f)
    o['eye80sq'] = _bf16(eye80sq)
    o['ones1'] = np.ones((1, P), f)
    o['ones32b'] = _bf16(np.ones((1, 32), f))
    return o


def _sigmoid(x):
    return 1.0 / (1.0 + np.exp(-x))


def numpy_forward(inp, t_steps=T, b_rows=B):
    """Numpy mirror of the kernel math (folded weights, permuted gates),
    in fp32 (no bf16 effects). Validates the host-side folds."""
    w = prep_weights(inp)
    x = np.asarray(inp['x'], np.float32)[:b_rows, :t_steps]
    U = {0: np.asarray(w['U0b'], np.float32),
         1: np.asarray(w['U1b'], np.float32),
         2: np.asarray(w['U2b'], np.float32)}
    W1 = np.asarray(w['W1b'], np.float32)
    W2 = np.asarray(w['W2b'], np.float32)

    def scan(xz, Um):
        bsz = xz.shape[0]
        h = np.zeros((bsz, H), np.float32)
        c = np.zeros((bsz, H), np.float32)
        hs = np.empty((bsz, t_steps, H), np.float32)
        for t in range(t_steps):
            z = xz[:, t] + h @ Um
            i = _sigmoid(z[:, _I_COLS]); f = _sigmoid(z[:, _F_COLS])
            o_ = _sigmoid(z[:, _O_COLS]); g = np.tanh(z[:, _G_COLS])
            c = f * c + i * g
            h = o_ * np.tanh(c)
            hs[:, t] = h
        return hs  # [B, T, H]

    xz0 = np.einsum('btd,dg->btg', x, w['W0p']) + w['b0row']
    h0 = scan(xz0, U[0])
    xz1 = np.einsum('bth,hg->btg', h0, W1) + np.asarray(w['b1b'], np.float32)
    h1 = scan(xz1, U[1])
    xz2 = np.einsum('bth,hg->btg', h1, W2) + np.asarray(w['b2b'], np.float32)
    h2 = scan(xz2, U[2])

    e = np.tanh(np.einsum('bth,hk->btk', h2, np.asarray(w['Wab'], np.float32))
                + w['ba'])
    s = e.sum(-1)
    s = s - s.max(axis=1, keepdims=True)
    a = np.exp(s); a = a / a.sum(axis=1, keepdims=True)
    pooled = np.einsum('bt,bth->bh', a, h2)
    d1 = np.maximum(pooled @ w['Wd1p'] + w['bd1'], 0)
    d2 = np.maximum(d1 @ w['Wd2p'] + w['bd2p'], 0)
    return d2 @ w['Wd3'] + w['bd3']


# ---------------------------------------------------------------------------
# Bass program
# ---------------------------------------------------------------------------

def build_nc(t_steps=T):
    import concourse.bacc as bacc
    import concourse.mybir as mybir
    import concourse.tile as tile
    from contextlib import ExitStack

    f32 = mybir.dt.float32
    f32r = mybir.dt.float32r
    bf16 = mybir.dt.bfloat16
    f8 = mybir.dt.float8e4
    AF = mybir.ActivationFunctionType
    OP = mybir.AluOpType
    PM = mybir.MatmulPerfMode
    M = t_steps * BL
    MT = M // P
    TPB = P // BL  # timesteps per 128-row tile (8)
    NSLOT = t_steps + LAG2

    nc = bacc.Bacc("TRN2", target_bir_lowering=False, debug=False,
                   num_devices=NC)

    def din(name, shape, dt=f32):
        return nc.dram_tensor(name, list(shape), dt, kind="ExternalInput")

    x_d = din('xT', (D, t_steps, BL))
    W0p = din('W0p', (D, G4)); b0row = din('b0row', (G4,))
    U_d = [din('U0b', (H, G4), f8), din('U1b', (H, G4), f8),
           din('U2b', (H, G4), f8)]
    W_d = {1: din('W1b', (H, G4), f8), 2: din('W2b', (H, G4), f8)}
    brow_d = {1: din('b1b', (G4,), bf16), 2: din('b2b', (G4,), bf16)}
    Wab = din('Wab', (H, H), f8); ba = din('ba', (H,))
    Wd1p = din('Wd1p', (H, P)); bd1 = din('bd1', (P,))
    Wd2p = din('Wd2p', (P, 64)); bd2p = din('bd2p', (64,))
    Wd3 = din('Wd3', (64, 5)); bd3 = din('bd3', (5,))
    sel_d = din('sel', (P, BL))
    ident_d = din('ident', (P, P))
    eye80sq_d = din('eye80sq', (80, 80), bf16)
    ones1_d = din('ones1', (1, P))
    ones32_d = din('ones32b', (1, 32), bf16)
    outT = nc.dram_tensor('outT', [5, BL], f32, kind="ExternalOutput")

    # DRAM temps
    xz_d = nc.dram_tensor('xz_buf', [M, G4], bf16)
    h2T = nc.dram_tensor('h2T', [4, P, t_steps, BL], f8)
    h2rows = nc.dram_tensor('h2rows', [M, H], f8)
    s_dram = nc.dram_tensor('s_dram', [M], f32)
    a_dram = nc.dram_tensor('a_dram', [M], f32)

    NSL = [slice(n * 512, (n + 1) * 512) for n in range(4)]
    ROWS = [slice(32 * l, 32 * l + BL) for l in range(3)]

    with tile.TileContext(nc) as tc:
        with ExitStack() as gctx:
            gconst = gctx.enter_context(tc.tile_pool(name="gconst", bufs=1))
            ident = gconst.tile([P, P], f32)
            nc.sync.dma_start(ident[:], ident_d[:, :])
            sel = gconst.tile([P, BL], f32)
            nc.sync.dma_start(sel[:], sel_d[:, :])

            # ---------------- layer-0 input projection pass ----------------
            def xz_pass():
                with ExitStack() as ctx:
                    cst = ctx.enter_context(tc.tile_pool(name="p0c", bufs=1))
                    W_stg = cst.tile([P, G4], f32, name="p0Ws")
                    nc.sync.dma_start(W_stg[:], W0p[:, :])
                    W_sb = cst.tile([P, G4], f32r, name="p0W")
                    nc.any.tensor_copy(W_sb[:], W_stg[:])
                    ones_s = cst.tile([1, P], f32, name="p0o_s")
                    nc.sync.dma_start(ones_s[:], ones1_d[:, :])
                    ones_sb = cst.tile([1, P], f32r, name="p0o")
                    nc.any.tensor_copy(ones_sb[:], ones_s[:])
                    b0_s = cst.tile([1, G4], f32, name="p0b_s")
                    nc.sync.dma_start(b0_s[:], b0row[None, :])
                    b0_sb = cst.tile([1, G4], f32r, name="p0b")
                    nc.any.tensor_copy(b0_sb[:], b0_s[:])
                    io = ctx.enter_context(tc.tile_pool(name="p0io", bufs=3))
                    ps = ctx.enter_context(
                        tc.tile_pool(name="p0ps", bufs=2, space="PSUM"))
                    for m in range(MT):
                        km_s = io.tile([P, P], f32, tag="km_s")
                        nc.sync.dma_start(
                            km_s[:].rearrange("p (t b) -> p t b", b=BL),
                            x_d[:, m * TPB:(m + 1) * TPB, :])
                        km = io.tile([P, P], f32r, tag="km")
                        nc.any.tensor_copy(km[:], km_s[:])
                        zp = ps.tile([P, G4], f32, tag="zp")
                        for n in range(4):
                            nc.tensor.matmul(zp[:, NSL[n]], ones_sb[:],
                                             b0_sb[:, NSL[n]],
                                             start=True, stop=False)
                            nc.tensor.matmul(zp[:, NSL[n]], km[:],
                                             W_sb[:, NSL[n]],
                                             start=False, stop=True)
                        ob = io.tile([P, G4], bf16, tag="ob")
                        nc.vector.tensor_copy(ob[:], zp[:])
                        nc.sync.dma_start(xz_d[m * P:(m + 1) * P, :], ob[:])

            # ---------------- 3-layer wavefront scan ----------------
            def wavefront():
                with ExitStack() as ctx:
                    cst = ctx.enter_context(tc.tile_pool(name="wfc", bufs=1))
                    U_sb = []
                    for l in range(3):
                        u = cst.tile([P, 4, G4], f8, name=f"wfU{l}")
                        nc.sync.dma_start(
                            u[:], U_d[l].rearrange("(k p) n -> p k n", p=P))
                        U_sb.append(u)
                    W_sb = {}
                    for l in (1, 2):
                        w = cst.tile([P, 4, G4], f8, name=f"wfW{l}")
                        nc.sync.dma_start(
                            w[:], W_d[l].rearrange("(k p) n -> p k n", p=P))
                        W_sb[l] = w
                    brow = {}
                    for l in (1, 2):
                        bt = cst.tile([1, G4], bf16, name=f"wfbr{l}")
                        nc.sync.dma_start(bt[:], brow_d[l][None, :])
                        brow[l] = bt
                    eye80 = cst.tile([80, 80], bf16, name="wfeye")
                    nc.sync.dma_start(eye80[:], eye80sq_d[:, :])
                    ones32 = cst.tile([1, 32], bf16, name="wfo32")
                    nc.sync.dma_start(ones32[:], ones32_d[:, :])
                    # persistent state: c split per chunk-half (tile-granular
                    # dep tracking — separate tiles avoid false WAR/RAW)
                    c_h = []
                    for hh in range(2):
                        t_ = cst.tile([80, 2, P], f32, name=f"wf_c{hh}")
                        nc.vector.memset(t_[:], 0.0)
                        c_h.append(t_)
                    # double-buffered (slot parity) per-half gate temps
                    cf_h = [[cst.tile([80, 2, P], f32, name=f"wf_cf{p}{hh}")
                             for hh in range(2)] for p in range(2)]
                    ig_h = [[cst.tile([80, 2, P], bf16, name=f"wf_ig{p}{hh}")
                             for hh in range(2)] for p in range(2)]
                    tch_h = [[cst.tile([80, 2, P], bf16, name=f"wf_tc{p}{hh}")
                              for hh in range(2)] for p in range(2)]
                    sig_h = [[cst.tile([80, 2, 384], bf16,
                                       name=f"wf_sg{p}{hh}")
                              for hh in range(2)] for p in range(2)]
                    gb_h = [[cst.tile([80, 2, P], bf16, name=f"wf_g{p}{hh}")
                             for hh in range(2)] for p in range(2)]
                    h_pp = []
                    tmp_pp = []
                    for p in range(2):
                        hb = cst.tile([96, H], f8, name=f"wf_h{p}")
                        nc.vector.memset(hb[:], 0.0)
                        h_pp.append(hb)
                        tb = cst.tile([96, H], f8, name=f"wf_tmp{p}")
                        nc.vector.memset(tb[:], 0.0)
                        tmp_pp.append(tb)
                    hT = [cst.tile([P, 4, HR, BL], f8, name=f"wfhT{l}")
                          for l in range(3)]
                    for l in range(3):
                        nc.vector.memset(hT[l][:], 0.0)
                    ring = cst.tile([80, RING, G4], bf16, name="wf_ring")
                    nc.vector.memset(ring[:], 0.0)
                    psp = ctx.enter_context(
                        tc.tile_pool(name="wfps", bufs=1, space="PSUM"))
                    # one PSUM tile per (parity, chunk-pair): 4 x 2 banks
                    Zpp = []
                    for p in range(2):
                        row = []
                        for hh in range(2):
                            t_ = psp.tile([P, 1024], f32,
                                          name=f"wf_z{p}{hh}")
                            nc.vector.memset(t_[:], 0.0)
                            row.append(t_)
                        Zpp.append(row)
                    wk = ctx.enter_context(tc.tile_pool(name="wfwk", bufs=3))

                    def prefetch_xz0(t0):
                        # load xz rows for steps [t0, t0+4) into ring rows 0:16
                        if 0 <= t0 < t_steps:
                            nt = min(4, t_steps - t0)
                            nc.gpsimd.dma_start(
                                ring[0:BL, t0 % RING:t0 % RING + nt, :],
                                xz_d[t0 * BL:(t0 + nt) * BL, :]
                                .rearrange("(t b) g -> b t g", b=BL))

                    prefetch_xz0(0)

                    SCAT_ENG = [nc.sync, nc.gpsimd, nc.scalar]
                    pj_state = {}  # live pj tile: (L, tau, stage)

                    def emit_pj_half(L, tau, hh, z):
                        """Emit pj matmuls for chunk pair hh into z rows
                        96:128 (DoubleRow fp8; evicted at slot end)."""
                        r0 = tau % HR
                        for q in range(2):
                            n = 2 * hh + q
                            nc.tensor.matmul(
                                z[hh][96:128, 512 * q:512 * (q + 1)],
                                ones32[:], brow[L][:, NSL[n]],
                                start=True, stop=False,
                                tile_position=(0, 96))
                        for k in range(4):
                            stat = hT[L - 1][:, k, r0:r0 + 2, :]
                            for q in range(2):
                                n = 2 * hh + q
                                nc.tensor.matmul(
                                    z[hh][96:128, 512 * q:512 * (q + 1)],
                                    stat, W_sb[L][:, k, NSL[n]],
                                    start=False, stop=(k == 3),
                                    tile_position=(0, 96))

                    ev_fin = None  # pj tile to CAST-evict + ring-store
                    ev_beg = None  # pj tile to IDENT-evict

                    for s in range(NSLOT):
                        ts_ = [s, s - LAG1, s - LAG2]
                        act = [0 <= t < t_steps for t in ts_]
                        rec = [act[l] and ts_[l] >= 1 for l in range(3)]
                        any_rec = any(rec)
                        r = s % RING
                        par = s % 2
                        z = Zpp[par]          # this slot's gate banks
                        sigb = sig_h[par]; gb = gb_h[par]; tch = tch_h[par]
                        cfp = cf_h[par]; igp = ig_h[par]
                        h_bf = h_pp[par]; tmp_bf = tmp_pp[par]

                        # ---- per-half tail: tanh(c), h, transpose ----
                        def tail(hh):
                            nc.scalar.activation(
                                tch[hh][:], c_h[hh][:], AF.Tanh)
                            nc.vector.tensor_tensor(
                                h_bf[0:80, 256 * hh:256 * (hh + 1)]
                                .rearrange("p (q c) -> p q c", q=2),
                                sigb[hh][:, :, 256:384], tch[hh][:],
                                OP.mult)
                            nc.vector.transpose(
                                tmp_bf[:, 256 * hh:256 * (hh + 1)],
                                h_bf[:, 256 * hh:256 * (hh + 1)])

                        # ---- per-half: inject + recurrent MMs + gates ----
                        for hh in range(2):
                            for q in range(2):
                                n = 2 * hh + q
                                nc.tensor.matmul(
                                    z[hh][0:80, 512 * q:512 * (q + 1)],
                                    eye80[:], ring[0:80, r, NSL[n]],
                                    start=True, stop=not any_rec,
                                    skip_group_check=True)
                                for k in range(4):
                                    for l in range(3):
                                        if rec[l]:
                                            nc.tensor.matmul(
                                                z[hh][32 * l:32 * l + BL,
                                                      512 * q:512 * (q + 1)],
                                                hT[l][:, k,
                                                      (ts_[l] - 1) % HR, :],
                                                U_sb[l][:, k, NSL[n]],
                                                start=False, stop=(k == 3),
                                                tile_position=(0, 32 * l),
                                                skip_group_check=True)
                            # gates for chunk pair hh
                            nc.scalar.activation(
                                sigb[hh][:],
                                z[hh][0:80, :]
                                .rearrange("p (q c) -> p q c", q=2)
                                [:, :, 0:384],
                                AF.Sigmoid)
                            nc.scalar.activation(
                                gb[hh][:],
                                z[hh][0:80, :]
                                .rearrange("p (q c) -> p q c", q=2)
                                [:, :, 384:512],
                                AF.Tanh)
                            nc.vector.tensor_tensor(
                                cfp[hh][:], c_h[hh][:],
                                sigb[hh][:, :, 128:256], OP.mult)
                            nc.gpsimd.tensor_tensor(
                                igp[hh][:], sigb[hh][:, :, 0:128],
                                gb[hh][:], OP.mult)
                            nc.vector.tensor_tensor(
                                c_h[hh][:], cfp[hh][:], igp[hh][:], OP.add)
                            if hh == 1:
                                tail(0)
                        tail(1)

                        # ---- pre-emit pj matmuls for slot s+1: they read
                        # only old hT ring positions, so issuing them BEFORE
                        # the scatters keeps the PE fed through the tail ----
                        z1 = Zpp[(s + 1) % 2]
                        fin1 = 2 if (s + 1) % 2 == 0 else 1
                        nf = pj_state.pop(fin1, None)
                        if nf is not None:
                            emit_pj_half(nf[0], nf[1], 1, z1)
                        beg1 = 1 if (s + 1) % 2 == 0 else 2
                        tau_b1 = s + 1 - 4 if beg1 == 1 else s + 1 - 17
                        nb = None
                        if 0 <= tau_b1 <= t_steps - 2:
                            stage = wk.tile([32, G4], bf16, tag="stage")
                            nb = (beg1, tau_b1, stage)
                            pj_state[beg1] = nb
                            emit_pj_half(beg1, tau_b1, 0, z1)

                        # ---- scatter transposed h into hT rings ----
                        for l in range(3):
                            if not act[l]:
                                continue
                            for j in range(4):
                                SCAT_ENG[(j + l) % 3].dma_start(
                                    hT[l][32 * j:32 * j + 32, :,
                                          ts_[l] % HR, :],
                                    tmp_bf[32 * l:32 * l + 32, :]
                                    .rearrange("p (k c) -> p k c", c=P)
                                    [:, :, 32 * j:32 * j + BL])

                        # ---- layer-2 outputs for attention ----
                        if act[2]:
                            t2 = ts_[2]
                            nc.sync.dma_start(
                                h2rows[t2 * BL:(t2 + 1) * BL, :],
                                h_bf[64:64 + BL, :])
                            if t2 % 4 == 3:
                                r4 = (t2 - 3) % HR
                                nc.sync.dma_start(
                                    h2T.rearrange("k p t b -> p k (t b)")
                                    [:, :, (t2 - 3) * BL:(t2 + 1) * BL],
                                    hT[2][:, :, r4:r4 + 4, :]
                                    .rearrange("p k t b -> p k (t b)"))

                        # ---- pj evictions for the MMs pre-emitted at the
                        # end of slot s-1 into this slot's parity tiles
                        # (lazy: those rows are rewritten 2 slots later) ----
                        if ev_fin is not None:
                            L, tau, stage = ev_fin
                            nc.vector.tensor_copy(
                                stage[:, G4 // 2:], z[1][96:128, :])
                            r0 = tau % HR
                            for i2 in range(2):
                                nc.scalar.dma_start(
                                    ring[32 * L:32 * L + BL, r0 + i2, :],
                                    stage[BL * i2:BL * (i2 + 1), :])
                        if ev_beg is not None:
                            nc.scalar.activation(
                                ev_beg[2][:, 0:G4 // 2], z[0][96:128, :],
                                AF.Identity)
                        ev_fin, ev_beg = nf, nb
                        if s % 4 == 0:
                            prefetch_xz0(s + 4)

            # ---------------- run pipeline ----------------
            xz_pass()
            wavefront()

            # ---------------- attention ----------------
            with ExitStack() as ctx:
                cst = ctx.enter_context(tc.tile_pool(name="atc", bufs=1))
                Wa_sb = cst.tile([P, 4, H], f8, name="atWa")
                nc.sync.dma_start(
                    Wa_sb[:], Wab.rearrange("(k p) n -> p k n", p=P))
                ba_rep = cst.tile([P, H], f32)
                nc.sync.dma_start(ba_rep[:], ba[None, :].to_broadcast((P, H)))
                s_sb = cst.tile([P, MT], f32)
                io = ctx.enter_context(tc.tile_pool(name="atio", bufs=3))
                ps = ctx.enter_context(
                    tc.tile_pool(name="atps", bufs=2, space="PSUM"))
                # e-pass: s[(t,b)] = sum_k tanh(h2 @ Wa + ba)
                for m in range(MT):
                    kxm = io.tile([P, 4, TPB, BL], f8, tag="kxm")
                    for k in range(4):
                        nc.sync.dma_start(
                            kxm[:, k],
                            h2T[k, :, m * TPB:(m + 1) * TPB, :])
                    ep = ps.tile([P, H], f32, tag="ep")
                    for k in range(4):
                        nc.tensor.matmul(
                            ep[:], kxm[:, k], Wa_sb[:, k, :],
                            start=(k == 0), stop=(k == 3))
                    e_sb = io.tile([P, H], f32, tag="e")
                    nc.vector.tensor_tensor(e_sb[:], ep[:], ba_rep[:], OP.add)
                    e_t = io.tile([P, H], f32, tag="et")
                    nc.scalar.activation(e_t[:], e_sb[:], AF.Tanh,
                                         accum_out=s_sb[:, m:m + 1])

                # s (row layout [P, MT]) -> sT [BL, t_steps] via flat DRAM
                nc.sync.dma_start(
                    s_dram.rearrange("(m p) -> p m", p=P), s_sb[:])
                sT = cst.tile([BL, t_steps], f32)
                nc.sync.dma_start(
                    sT[:], s_dram.rearrange("(t b) -> b t", b=BL))
                mx = cst.tile([BL, 1], f32)
                nc.vector.reduce_max(mx[:], sT[:], axis=mybir.AxisListType.X)
                nmx = cst.tile([BL, 1], f32)
                nc.vector.tensor_scalar_mul(nmx[:], mx[:], -1.0)
                ex = cst.tile([BL, t_steps], f32)
                sm = cst.tile([BL, 1], f32)
                nc.scalar.activation(ex[:], sT[:], AF.Exp, bias=nmx[:],
                                     accum_out=sm[:])
                rs = cst.tile([BL, 1], f32)
                nc.vector.reciprocal(rs[:], sm[:])
                aT = cst.tile([BL, t_steps], f32)
                nc.vector.tensor_scalar_mul(aT[:], ex[:], rs[:])
                nc.sync.dma_start(
                    a_dram.rearrange("(t b) -> b t", b=BL), aT[:])
                a_row = cst.tile([P, MT], f32)
                nc.sync.dma_start(
                    a_row[:], a_dram.rearrange("(m p) -> p m", p=P))

                # pooled[b, :] = sum_rows sel * (a * h2)   (f32 matmuls)
                pp = ctx.enter_context(
                    tc.tile_pool(name="atpp", bufs=1, space="PSUM"))
                ps1 = ctx.enter_context(
                    tc.tile_pool(name="atp1", bufs=1, space="PSUM"))
                pooled_ps = pp.tile([BL, H], f32)
                for m in range(MT):
                    h2t = io.tile([P, H], f8, tag="h2t")
                    nc.sync.dma_start(h2t[:], h2rows[m * P:(m + 1) * P, :])
                    wrow = io.tile([P, H], f32, tag="wrow")
                    nc.vector.tensor_scalar_mul(wrow[:], h2t[:],
                                                a_row[:, m:m + 1])
                    nc.tensor.matmul(pooled_ps[:], sel[:], wrow[:],
                                     start=(m == 0), stop=(m == MT - 1))

                # pooledT via PE transpose
                pooled_sb = cst.tile([BL, H], f32)
                nc.vector.tensor_copy(pooled_sb[:], pooled_ps[:])
                ptp = ps1.tile([P, 4 * BL], f32, tag="ptp")
                for k in range(4):
                    nc.tensor.transpose(
                        ptp[:, k * BL:(k + 1) * BL],
                        pooled_sb[:, k * P:(k + 1) * P], ident[0:BL, 0:BL])
                pooledT = cst.tile([P, 4, BL], f32r)
                nc.vector.tensor_copy(
                    pooledT[:], ptp[:].rearrange("p (k b) -> p k b", k=4))

                # ---------------- dense head ----------------
                def load_r(pool, dram_ap, shape, name):
                    stg = pool.tile(shape, f32, name=name + "_stg")
                    nc.sync.dma_start(stg[:], dram_ap)
                    t_ = pool.tile(shape, f32r, name=name)
                    nc.any.tensor_copy(t_[:], stg[:])
                    return t_

                Wd1_sb = load_r(cst, Wd1p.rearrange("(k p) n -> p k n", p=P),
                                [P, 4, P], "hWd1")
                bd1_sb = cst.tile([P, 1], f32)
                nc.sync.dma_start(bd1_sb[:], bd1[:, None])
                Wd2_sb = load_r(cst, Wd2p[:, :], [P, 64], "hWd2")
                bd2_sb = cst.tile([64, 1], f32)
                nc.sync.dma_start(bd2_sb[:], bd2p[:, None])
                Wd3_sb = load_r(cst, Wd3[:, :], [64, 5], "hWd3")
                bd3_sb = cst.tile([5, 1], f32)
                nc.sync.dma_start(bd3_sb[:], bd3[:, None])

                d1p = ps1.tile([P, BL], f32, tag="d1p")
                for k in range(4):
                    nc.tensor.matmul(d1p[:], Wd1_sb[:, k, :], pooledT[:, k, :],
                                     start=(k == 0), stop=(k == 3))
                d1 = cst.tile([P, BL], f32r)
                nc.scalar.activation(d1[:], d1p[:], AF.Relu, bias=bd1_sb[:])
                d2p = ps1.tile([64, BL], f32, tag="d2p")
                nc.tensor.matmul(d2p[:], Wd2_sb[:], d1[:], start=True,
                                 stop=True)
                d2 = cst.tile([64, BL], f32r)
                nc.scalar.activation(d2[:], d2p[:], AF.Relu, bias=bd2_sb[:])
                d3p = ps1.tile([5, BL], f32, tag="d3p")
                nc.tensor.matmul(d3p[:], Wd3_sb[:], d2[:], start=True,
                                 stop=True)
                d3 = cst.tile([5, BL], f32)
                nc.scalar.activation(d3[:], d3p[:], AF.Identity, bias=bd3_sb[:])
                nc.sync.dma_start(outT[:, :], d3[:])

    nc.compile()
    return nc


@functools.lru_cache(maxsize=2)
def _compiled(t_steps):
    return build_nc(t_steps)


def _make_in_maps(inputs):
    w = prep_weights(inputs)
    x = np.ascontiguousarray(np.asarray(inputs['x'], np.float32))
    base = {k: w[k] for k in (
        'W0p', 'b0row', 'U0b', 'U1b', 'U2b', 'W1b', 'b1b', 'W2b', 'b2b',
        'Wab', 'ba', 'Wd1p', 'bd1', 'Wd2p', 'bd2p', 'Wd3', 'bd3', 'sel',
        'ident', 'eye80sq', 'ones1', 'ones32b')}
    in_maps = []
    for c in range(NC):
        m = dict(base)
        m['xT'] = np.ascontiguousarray(
            x[c * BL:(c + 1) * BL].transpose(2, 1, 0))
        in_maps.append(m)
    return in_maps


def kernel(**inputs):
    from concourse import bass_utils
    nc = _compiled(T)
    in_maps = _make_in_maps(inputs)
    res = bass_utils.run_bass_kernel_spmd(nc, in_maps, core_ids=list(range(NC)))
    out = np.concatenate([np.asarray(res.results[c]['outT']).T
                          for c in range(NC)], axis=0)
    return np.ascontiguousarray(out, np.float32)


def timed_run(tmpdir=None, **inputs):
    """Run with NTFF profiling; returns BassKernelResults."""
    from concourse import bass_utils
    nc = _compiled(T)
    in_maps = _make_in_maps(inputs)
    res = bass_utils.run_bass_kernel_spmd(
        nc, in_maps, core_ids=list(range(NC)), trace=True, tmpdir=tmpdir)
    return res

